# revision 24
# baseline (speedup 1.0000x reference)
"""Causal self-attention (B=4, T=2048, C=1024, H=16, D=64) on 8 trn2 cores.

Sharding: data-parallel over B (4) x tensor-parallel over head-halves (2).
Core c handles batch c//2 with heads [8*(c%2), 8*(c%2)+8). Each core emits a
partial projection output [2048, 1024]; host sums the two head-half partials
per batch and adds the (bv @ Wp + bp) correction row.

Device layout highlights:
 - QKV projections run as fp8e4m3 DoubleRow matmuls (2 k-tiles per pass, 0.5
   cycles/row) with a 3-term residual split prepared on the host:
   x@w ~= x_hi@w_hi + x_lo@w_hi + x_hi@w_lo, each operand quantized e4m3.
   This is 3/4 the PE cost of fp32r at ~5e-3 max rel err per GEMM.
 - everything downstream of the QKV psums is bf16: Q^T/K^T/V tiles, exp(S)
   tiles, O^T tiles and Wp. bf16 matmuls run at 1.0 cycles/row at ANY width
   (fp32r pays 4x below 256), halve SBUF footprint and DMA bytes, and cost
   ~0.3% relative error against a 2e-2 budget.
 - S^T = K^T.T @ Q^T keeps softmax denominators computable by an in-matmul
   ones-column (V' has a 65th column of ones -> row 64 of O' = Z)
 - softmax skips max-subtraction (logits are ~N(0,1); exp cannot overflow)
 - causal masking via 0/1 mask multiply on the diagonal-block patterns,
   executed on the Pool engine (otherwise idle)
 - softmax denominators: DVE reciprocal of the Z row + Pool-engine
   partition_broadcast (replaces the former ones-column PE matmul)
 - attention processes two heads in lockstep so the PE never waits on the
   Act engine's exp: S_a, S_b, PV_a, S_a', PV_b, ...; QKV/projection work is
   woven between head-pairs as PE filler while Act drains exps
 - projection runs as 8 K=64 groups against per-head-half [64,512] O^T tiles,
   so both heads' normalize muls write partition-0-based tiles and the old
   partition-shifting SBUF->SBUF DMA disappears
"""

import os
import sys

for _p in ("/opt/trn_rl_repo", "/root/.axon_site/_ro/trn_rl_repo"):
    if os.path.isdir(_p) and _p not in sys.path:
        sys.path.insert(0, _p)

import ml_dtypes
import numpy as np
from concourse import bacc, mybir, tile
from concourse.bass_utils import run_bass_kernel_spmd

N_CORES = 8
B, T, C = 4, 2048, 1024
H, D = 16, 64          # full model heads
HG = 8                 # heads per core (head-group)
CH = HG * D            # 512, per-core qkv width
NT = T // 128          # 16 s-tiles
NJ = T // 512          # 4 t-chunks
NP = C // 256          # 4 DoubleRow contraction pairs
F32 = mybir.dt.float32
F32R = mybir.dt.float32r
BF16 = mybir.dt.bfloat16
F8 = mybir.dt.float8e4
DRM = mybir.MatmulPerfMode.DoubleRow
AF = mybir.ActivationFunctionType
FP8 = ml_dtypes.float8_e4m3

# (x_half, w_half) residual terms; term-major so the hi*hi sweep only needs
# the hi tiles that arrive first, then x-lo (scalar queue) before w-lo (sync)
TERMS = ((0, 0), (1, 0), (0, 1))
NTERM = len(TERMS)

_CACHE = {}


def _emit(nc, tc, aps):
    xq, wq, wk, wv, wp, bq2, bk2, mask, yout = (
        aps["xq"], aps["wq"], aps["wk"], aps["wv"], aps["wp"],
        aps["bq2"], aps["bk2"], aps["mask"], aps["y"],
    )

    pool = tc.alloc_tile_pool(name="pool", bufs=1)
    psp = tc.alloc_tile_pool(name="ps", bufs=1, space="PSUM")

    # ---- persistent tensors ----
    kt = [pool.tile([128, T], BF16, name=f"kt{m}", tag="kt", bufs=4)
          for m in range(4)]
    vp = [pool.tile([128, 520], BF16, name=f"vp{i}", tag="vp", bufs=NT)
          for i in range(NT)]
    # single lower-triangle mask (1{s <= t}) for the diagonal 128x128 blocks
    tri = pool.tile([128, 128], BF16, name="tri", tag="tri", bufs=1)
    bqs = pool.tile([128, 4], F32, name="bqs", tag="bias", bufs=2)
    bks = pool.tile([128, 4], F32, name="bks", tag="bias", bufs=2)
    ones_f = pool.tile([128, 64], F32, name="ones_f", tag="ones_f", bufs=1)

    # fp8 DoubleRow weights: [128, pair, s, 512] hi and lo tiles per matrix
    wqkv = {}
    for nm in ("wq", "wk", "wv"):
        wqkv[nm] = [pool.tile([128, NP, 2, CH], F8, name=f"{nm}{hl}",
                              tag="w8", bufs=6) for hl in range(2)]
    # chunk-0 x, split hi/lo for a fast start; later chunks combined
    x0 = [pool.tile([128, NP, 2, 512], F8, name=f"x0_{hl}", tag="x0", bufs=2)
          for hl in range(2)]
    xcomb = [None] * NJ  # chunks 1..3: [128, hl, pair, s, 512] tiles

    # DMA plan: sync carries wq/wk and all later x chunks; scalar carries the
    # chunk-0 x, biases, mask and wv, then stays idle so the Act engine's
    # sequencer is free once the exp stream spins up.
    nc.sync.dma_start(wqkv["wq"][0][:, 0], wq[:, 0, 0])
    nc.scalar.dma_start(x0[0][:, 0], xq[:, 0, 0, 0])
    nc.sync.dma_start(wqkv["wq"][0][:, 1:4], wq[:, 0, 1:4])
    nc.scalar.dma_start(x0[0][:, 1:4], xq[:, 0, 0, 1:4])
    nc.sync.dma_start(wqkv["wq"][1][:], wq[:, 1])
    nc.scalar.dma_start(x0[1][:], xq[:, 0, 1])
    nc.sync.dma_start(wqkv["wk"][0][:], wk[:, 0])
    nc.scalar.dma_start(wqkv["wv"][0][:], wv[:, 0])
    nc.scalar.dma_start(bqs[:], bq2[:])
    nc.scalar.dma_start(bks[:], bk2[:])
    nc.sync.dma_start(wqkv["wk"][1][:], wk[:, 1])
    nc.scalar.dma_start(tri[:], mask[:])
    nc.scalar.dma_start(wqkv["wv"][1][:], wv[:, 1])
    nc.gpsimd.memset(ones_f[:], 1.0)
    for i in range(NT):
        ocol = vp[i][:, 0:520].rearrange("p (h e) -> p h e", e=65)[:, :, 64:65]
        nc.vector.tensor_copy(ocol, ones_f[:, 0:8].unsqueeze(2))

    def load_x(j):
        xt_t = pool.tile([128, 2, NP, 2, 512], F8, name=f"xt{j}", tag="xt",
                         bufs=2)
        nc.sync.dma_start(xt_t[:], xq[:, j])
        xcomb[j] = xt_t

    def x_ap(j, p, xh, c0, cw):
        if j == 0:
            return x0[xh][:, p, :, c0:c0 + cw]
        return xcomb[j][:, xh, p, :, c0:c0 + cw]

    qtc = [[None] * NJ for _ in range(4)]   # per-chunk Q^T tiles
    otc = [[None] * NJ for _ in range(4)]   # per-chunk O^T tiles
    wps = [[None, None] for _ in range(4)]  # wp [128,512] halves, loaded late

    def _qkv_psum(idx, j, nm):
        # chunk 0 runs all four blocks of a part concurrently, term-staged,
        # so the PE can sweep the hi*hi term as soon as the hi tiles land;
        # the sp-tag PSUM slots are idle during chunk 0, borrow two of them
        if j == 0 and idx >= 2:
            return psp.tile([128, 1024], F32, name=nm, tag="sp",
                            bufs=2)[:, 0:512]
        return psp.tile([128, 512], F32, name=nm, tag="qk", bufs=2)[:]

    def emit_qkv(j, parts="qkv", sel=(0, 1, 2, 3)):
        # Q^T and K^T: out [128 qk-dims, 512 t] per m-block
        for part in parts:
            if part in "qk":
                wsrc = wqkv["wq"] if part == "q" else wqkv["wk"]
                bias_t = bqs if part == "q" else bks
                pss = [_qkv_psum(i, j, f"{part}ps{j}_{m}")
                       for i, m in enumerate(sel)]
                for ti, (xh, wh) in enumerate(TERMS):
                    for i, m in enumerate(sel):
                        for u in range(2):
                            for p in range(NP):
                                nc.tensor.matmul(
                                    pss[i][:, 256 * u:256 * u + 256],
                                    wsrc[wh][:, p, :, 128 * m:128 * m + 128],
                                    x_ap(j, p, xh, 256 * u, 256),
                                    start=(ti == 0 and p == 0),
                                    stop=(ti == NTERM - 1 and p == NP - 1),
                                    perf_mode=DRM,
                                )
                for i, m in enumerate(sel):
                    if part == "q":
                        t_ = pool.tile([128, 512], BF16, name=f"qt{m}_{j}",
                                       tag="qtc", bufs=8)
                        qtc[m][j] = t_
                        out_ap = t_[:]
                    else:
                        out_ap = kt[m][:, 512 * j:512 * j + 512]
                    # chunk 0: DVE is idle; later chunks run inside attention
                    # where DVE carries the normalize chains, so Pool moves it
                    eng = nc.vector if j == 0 else nc.gpsimd
                    eng.tensor_scalar_add(out_ap, pss[i], bias_t[:, m:m + 1])
            else:
                # V: out [128 t-slice, 512 v-dims]
                pss = [_qkv_psum(i, j, f"vps{4 * j + u}")
                       for i, u in enumerate(sel)]
                for ti, (xh, wh) in enumerate(TERMS):
                    for i, u in enumerate(sel):
                        for h2 in range(2):
                            for p in range(NP):
                                nc.tensor.matmul(
                                    pss[i][:, 256 * h2:256 * h2 + 256],
                                    x_ap(j, p, xh, 128 * u, 128),
                                    wqkv["wv"][wh][:, p, :,
                                                   256 * h2:256 * h2 + 256],
                                    start=(ti == 0 and p == 0),
                                    stop=(ti == NTERM - 1 and p == NP - 1),
                                    perf_mode=DRM,
                                )
                for i, u in enumerate(sel):
                    dst = vp[4 * j + u][:, 0:520].rearrange(
                        "p (h e) -> p h e", e=65)[:, :, 0:64]
                    src = pss[i].rearrange("p (h e) -> p h e", e=64)
                    eng = nc.vector if j == 0 else nc.gpsimd
                    eng.tensor_copy(dst, src)

    def tile_layout(p, j):
        # pairs of s-tiles per [128,1024] PSUM slot; diagonal tiles are
        # narrowed to the causally valid t-range [128r, 512).
        # entries: (i, slot_col, valid_t0, width, diag_block_col)
        i0, i1 = 2 * p, 2 * p + 1
        r0_, r1_ = i0 - 4 * j, i1 - 4 * j
        if r1_ < 0:
            return [(i0, 0, 0, 512, None), (i1, 512, 0, 512, None)], 1024
        if r0_ == 0:
            return [(i0, 0, 0, 512, 0), (i1, 512, 128, 384, 512)], 896
        return [(i0, 0, 256, 256, 0), (i1, 256, 384, 128, 256)], 384

    def emit_attn(j, mts=(0, 1, 2, 3), filler=None):
        # process the two heads of each mt pair in lockstep: the PE alternates
        # S and PV between the heads, so each head's exp runs while the other
        # head's matmul occupies the PE. `filler` emits PE work between mts
        # while the Act engine catches up on exps.
        n_i = 4 * j + 4
        npairs = n_i // 2
        for mt in mts:
            hA, hB = 2 * mt, 2 * mt + 1

            ops = {}
            ets = {}
            for h in (hA, hB):
                ops[h] = psp.tile([65, 512], F32, name=f"ops{h}_{j}", tag="o",
                                  bufs=2)

            def emit_s(h, p):
                off = 64 * (h % 2)
                layout, exp_hi = tile_layout(p, j)
                sp = psp.tile([128, 1024], F32, name=f"sp{h}_{j}_{p}", tag="sp",
                              bufs=2)
                for (i, scol, t0, w, _) in layout:
                    nc.tensor.matmul(
                        sp[:, scol:scol + w],
                        kt[mt][off:off + 64, 128 * i:128 * i + 128],
                        qtc[mt][j][off:off + 64, t0:t0 + w],
                        start=True, stop=True,
                    )
                et = pool.tile([128, 1024], BF16, name=f"et{h}_{j}_{p}",
                               tag="et", bufs=4)
                nc.scalar.activation(et[:, 0:exp_hi], sp[:, 0:exp_hi], AF.Exp,
                                     scale=0.125)
                for (i, scol, t0, w, dcol) in layout:
                    if dcol is not None:
                        blk = et[:, dcol:dcol + 128]
                        nc.gpsimd.tensor_mul(blk, blk, tri[:])
                ets[h] = (et, layout)

            def emit_pv(h, p):
                et, layout = ets[h]
                for (i, scol, t0, w, _) in layout:
                    nc.tensor.matmul(
                        ops[h][:, t0:t0 + w], vp[i][:, 65 * h:65 * h + 65],
                        et[:, scol:scol + w],
                        start=(i == 0), stop=(i == n_i - 1),
                    )

            # software pipeline across the two heads
            emit_s(hA, 0)
            for p in range(npairs):
                emit_s(hB, p)
                emit_pv(hA, p)
                if p + 1 < npairs:
                    emit_s(hA, p + 1)
                emit_pv(hB, p)

            # normalize: rows 0..63 unnormalized O^T, row 64 = Z
            # 1/Z on one partition, Pool broadcasts it across the 64 O rows
            rbs = {}
            for h in (hA, hB):
                rb1 = pool.tile([1, 512], F32R, name=f"rb1{h}_{j}", tag="rb1",
                                bufs=2)
                with nc.allow_low_precision(reason="fp32r softmax denom"):
                    nc.vector.reciprocal(rb1[:], ops[h][64:65, :])
                rbs_t = pool.tile([64, 512], F32R, name=f"rbs{h}_{j}",
                                  tag="rbs", bufs=2)
                nc.gpsimd.partition_broadcast(rbs_t[:], rb1[:])
                rbs[h] = rbs_t
            # all 16 O^T tiles stay live until their chunk's projection
            if otc[mt][j] is None:
                otc[mt][j] = pool.tile([128, 512], BF16, name=f"ot{mt}_{j}",
                                       tag="otc", bufs=16)
            nc.vector.tensor_mul(otc[mt][j][0:64, :], ops[hA][0:64, :],
                                 rbs[hA][:])
            st = pool.tile([64, 512], BF16, name=f"st{hB}_{j}", tag="st",
                           bufs=2)
            nc.vector.tensor_mul(st[:], ops[hB][0:64, :], rbs[hB][:])
            # scalar queue: idle mid-kernel, so the shift never queues behind
            # x/weight/output traffic on sync
            nc.scalar.dma_start(otc[mt][j][64:128, :], st[:])
            if filler:
                filler.pop(0)()

    def emit_wp_loads():
        for m in range(4):
            for n in range(2):
                t_ = pool.tile([128, 512], BF16, name=f"wps{m}_{n}",
                               tag="wp2", bufs=8)
                wps[m][n] = t_
                nc.sync.dma_start(
                    t_[:],
                    wp[128 * m:128 * m + 128, 512 * n:512 * n + 512],
                )

    def emit_proj(j, us=(0, 1, 2, 3), tail=False):
        # yo copies alternate DVE / Pool to spread the drain work; the
        # post-attention tail uses the freed "o" PSUM ring for double slots
        # and the now-idle Act engine for copies
        for u in us:
            t = 4 * j + u
            for n in range(2):
                tag = "o" if (tail and (u + n) % 2 == 0) else "qk"
                ps = psp.tile([128, 512], F32, name=f"yps{t}_{n}", tag=tag,
                              bufs=2)
                for m in range(4):
                    nc.tensor.matmul(
                        ps[:], otc[m][j][:, 128 * u:128 * u + 128],
                        wps[m][n][:],
                        start=(m == 0), stop=(m == 3),
                    )
                yo = pool.tile([128, 512], F32, name=f"yo{t}_{n}", tag="yo",
                               bufs=4)
                if tail:
                    nc.scalar.copy(yo[:], ps[:])
                else:
                    eng = nc.vector if (u + n) % 2 == 0 else nc.gpsimd
                    eng.tensor_copy(yo[:], ps[:])
                dma_eng = nc.scalar if (tail and (u + n) % 2 == 1) else nc.sync
                dma_eng.dma_start(
                    yout[128 * t:128 * t + 128, 512 * n:512 * n + 512], yo[:]
                )

    # ---- schedule ----
    # qkv(j+1) is woven between attn(j)'s head-pairs as PE filler while the
    # Act engine catches up on the exp backlog; attn(3) gets the projections.
    def rest(jn, *extra):
        def f():
            emit_qkv(jn, parts="q", sel=(2, 3))
            emit_qkv(jn, parts="k", sel=(2, 3))
            emit_qkv(jn, parts="v", sel=(2, 3))
            for e in extra:
                e()
        return f

    emit_qkv(0)
    load_x(1)
    emit_attn(0, filler=[lambda: emit_qkv(1, parts="q", sel=(0, 1)),
                         lambda: emit_qkv(1, parts="k", sel=(0, 1)),
                         lambda: emit_qkv(1, parts="v", sel=(0, 1)),
                         rest(1, lambda: load_x(2), emit_wp_loads)])
    emit_attn(1, filler=[lambda: emit_qkv(2, parts="q", sel=(0, 1)),
                         lambda: emit_qkv(2, parts="k", sel=(0, 1)),
                         lambda: emit_qkv(2, parts="v", sel=(0, 1)),
                         rest(2, lambda: load_x(3))])
    emit_attn(2, filler=[lambda: emit_qkv(3, parts="q", sel=(0, 1)),
                         lambda: emit_qkv(3, parts="k", sel=(0, 1)),
                         lambda: emit_qkv(3, parts="v", sel=(0, 1)),
                         rest(3)])
    emit_attn(3, filler=[lambda: emit_proj(0, us=(0, 1)),
                         lambda: emit_proj(0, us=(2, 3)),
                         lambda: emit_proj(1, us=(0, 1)),
                         lambda: None])
    emit_proj(1, us=(2, 3), tail=True)
    emit_proj(2, tail=True)
    emit_proj(3, tail=True)

    for m in range(4):
        qtc[m] = [None] * NJ
        otc[m] = [None] * NJ
    pool.release()
    psp.release()


def build(passes=1):
    key = ("nc", passes)
    if key in _CACHE:
        return _CACHE[key]
    nc = bacc.Bacc("TRN2", target_bir_lowering=False, debug=False,
                   num_devices=N_CORES)
    aps = {
        # fp8 DoubleRow operands, host-packed hl-major so chunk/pair slices
        # stay contiguous: contraction row = 256*pair + 128*s + partition,
        # hl = hi/lo residual half
        "xq": nc.dram_tensor("xq", [128, NJ, 2, NP, 2, 512], F8,
                             kind="ExternalInput").ap(),
        "wq": nc.dram_tensor("wq", [128, 2, NP, 2, CH], F8,
                             kind="ExternalInput").ap(),
        "wk": nc.dram_tensor("wk", [128, 2, NP, 2, CH], F8,
                             kind="ExternalInput").ap(),
        "wv": nc.dram_tensor("wv", [128, 2, NP, 2, CH], F8,
                             kind="ExternalInput").ap(),
        "wp": nc.dram_tensor("wp", [CH, C], BF16, kind="ExternalInput").ap(),
        "bq2": nc.dram_tensor("bq2", [128, 4], F32, kind="ExternalInput").ap(),
        "bk2": nc.dram_tensor("bk2", [128, 4], F32, kind="ExternalInput").ap(),
        "mask": nc.dram_tensor("mask", [128, 128], BF16,
                               kind="ExternalInput").ap(),
        "y": nc.dram_tensor("y", [T, C], F32, kind="ExternalOutput").ap(),
    }
    with tile.TileContext(nc) as tc:
        for _ in range(passes):
            _emit(nc, tc, aps)
    nc.compile()
    _CACHE[key] = nc
    return nc


def _pack_w(a):
    """[1024, CH] f32 -> [128, 2(hl), NP, 2(s), CH] fp8 DoubleRow pack
    with hi/lo residual split."""
    r = a.reshape(NP, 2, 128, CH)              # (pair, s, p, cols)
    hi = r.astype(FP8)
    lo = (r - hi.astype(np.float32)).astype(FP8)
    out = np.stack([hi, lo], axis=2)           # (pair, s, hl, p, cols)
    return np.ascontiguousarray(out.transpose(3, 2, 0, 1, 4))


def _pack_x(a):
    """[1024, T] f32 -> [128, NJ, 2(hl), NP, 2(s), 512] fp8 DoubleRow pack."""
    r = a.reshape(NP, 2, 128, NJ, 512)         # (pair, s, p, j, tl)
    hi = r.astype(FP8)
    lo = (r - hi.astype(np.float32)).astype(FP8)
    out = np.stack([hi, lo], axis=2)           # (pair, s, hl, p, j, tl)
    return np.ascontiguousarray(out.transpose(3, 4, 2, 0, 1, 5))


def make_in_maps(x, Wq, bq, Wk, bk, Wv, bv, Wp, bp):
    # lower-triangle 0/1 mask for the diagonal 128x128 attention blocks
    s_idx = np.arange(128)[:, None]
    t_idx = np.arange(128)[None, :]
    mask = (s_idx <= t_idx).astype(ml_dtypes.bfloat16)
    in_maps = []
    for c in range(N_CORES):
        b, g = c // 2, c % 2
        cols = slice(CH * g, CH * g + CH)
        in_maps.append({
            "xq": _pack_x(np.ascontiguousarray(x[b].T)),
            "wq": _pack_w(np.ascontiguousarray(Wq[:, cols])),
            "wk": _pack_w(np.ascontiguousarray(Wk[:, cols])),
            "wv": _pack_w(np.ascontiguousarray(Wv[:, cols])),
            "wp": np.ascontiguousarray(Wp[cols, :]).astype(ml_dtypes.bfloat16),
            "bq2": np.ascontiguousarray(bq[cols].reshape(4, 128).T),
            "bk2": np.ascontiguousarray(bk[cols].reshape(4, 128).T),
            "mask": np.ascontiguousarray(mask),
        })
    return in_maps


def kernel(x, Wq, bq, Wk, bk, Wv, bv, Wp, bp):
    # host-side prep is pure numpy; convert in case jax arrays are passed
    x, Wq, bq, Wk, bk, Wv, bv, Wp, bp = (
        np.asarray(a, dtype=np.float32)
        for a in (x, Wq, bq, Wk, bk, Wv, bv, Wp, bp)
    )
    nc = build()
    in_maps = make_in_maps(x, Wq, bq, Wk, bk, Wv, bv, Wp, bp)
    # the axon-proxied device occasionally reports a transient unrecoverable
    # exec state that clears on a fresh attempt; retry rather than fail
    last_err = None
    for _attempt in range(3):
        try:
            res = run_bass_kernel_spmd(nc, in_maps, core_ids=list(range(N_CORES)))
            break
        except Exception as e:  # noqa: BLE001
            last_err = e
            import time as _time
            _time.sleep(5)
    else:
        raise last_err
    corr = (bv @ Wp + bp).astype(np.float32)
    out = np.empty((B, T, C), dtype=np.float32)
    for b in range(B):
        out[b] = res.results[2 * b]["y"] + res.results[2 * b + 1]["y"] + corr
    return out


# revision 25
# speedup vs baseline: 1.0273x; 1.0273x over previous
"""Causal self-attention (B=4, T=2048, C=1024, H=16, D=64) on 8 trn2 cores.

Sharding: data-parallel over B (4) x tensor-parallel over head-halves (2).
Core c handles batch c//2 with heads [8*(c%2), 8*(c%2)+8). Each core emits a
partial projection output [2048, 1024]; host sums the two head-half partials
per batch and adds the (bv @ Wp + bp) correction row.

Device layout highlights:
 - QKV projections run as fp8e4m3 DoubleRow matmuls (2 k-tiles per pass, 0.5
   cycles/row) with a 3-term residual split prepared on the host:
   x@w ~= x_hi@w_hi + x_lo@w_hi + x_hi@w_lo, each operand quantized e4m3.
   This is 3/4 the PE cost of fp32r at ~5e-3 max rel err per GEMM.
 - everything downstream of the QKV psums is bf16: Q^T/K^T/V tiles, exp(S)
   tiles, O^T tiles and Wp. bf16 matmuls run at 1.0 cycles/row at ANY width
   (fp32r pays 4x below 256), halve SBUF footprint and DMA bytes, and cost
   ~0.3% relative error against a 2e-2 budget.
 - S^T = K^T.T @ Q^T keeps softmax denominators computable by an in-matmul
   ones-column (V' has a 65th column of ones -> row 64 of O' = Z)
 - softmax skips max-subtraction (logits are ~N(0,1); exp cannot overflow)
 - causal masking via 0/1 mask multiply on the diagonal-block patterns,
   executed on the Pool engine (otherwise idle)
 - softmax denominators: DVE reciprocal of the Z row + Pool-engine
   partition_broadcast (replaces the former ones-column PE matmul)
 - attention processes two heads in lockstep so the PE never waits on the
   Act engine's exp: S_a, S_b, PV_a, S_a', PV_b, ...; QKV/projection work is
   woven between head-pairs as PE filler while Act drains exps
 - projection runs as 8 K=64 groups against per-head-half [64,512] O^T tiles,
   so both heads' normalize muls write partition-0-based tiles and the old
   partition-shifting SBUF->SBUF DMA disappears
"""

import os
import sys

for _p in ("/opt/trn_rl_repo", "/root/.axon_site/_ro/trn_rl_repo"):
    if os.path.isdir(_p) and _p not in sys.path:
        sys.path.insert(0, _p)

import ml_dtypes
import numpy as np
from concourse import bacc, mybir, tile
from concourse.bass_utils import run_bass_kernel_spmd

N_CORES = 8
B, T, C = 4, 2048, 1024
H, D = 16, 64          # full model heads
HG = 8                 # heads per core (head-group)
CH = HG * D            # 512, per-core qkv width
NT = T // 128          # 16 s-tiles
NJ = T // 512          # 4 t-chunks
NP = C // 256          # 4 DoubleRow contraction pairs
F32 = mybir.dt.float32
F32R = mybir.dt.float32r
BF16 = mybir.dt.bfloat16
F8 = mybir.dt.float8e4
DRM = mybir.MatmulPerfMode.DoubleRow
AF = mybir.ActivationFunctionType
FP8 = ml_dtypes.float8_e4m3

# (x_half, w_half) residual terms; term-major so the hi*hi sweep only needs
# the hi tiles that arrive first, then x-lo (scalar queue) before w-lo (sync)
TERMS = ((0, 0), (1, 0), (0, 1))
NTERM = len(TERMS)

_CACHE = {}


def _emit(nc, tc, aps):
    xq, wq, wk, wv, wp, bq2, bk2, mask, yout = (
        aps["xq"], aps["wq"], aps["wk"], aps["wv"], aps["wp"],
        aps["bq2"], aps["bk2"], aps["mask"], aps["y"],
    )

    pool = tc.alloc_tile_pool(name="pool", bufs=1)
    psp = tc.alloc_tile_pool(name="ps", bufs=1, space="PSUM")

    # ---- persistent tensors ----
    kt = [pool.tile([128, T], BF16, name=f"kt{m}", tag="kt", bufs=4)
          for m in range(4)]
    vp = [pool.tile([128, 520], BF16, name=f"vp{i}", tag="vp", bufs=NT)
          for i in range(NT)]
    # single lower-triangle mask (1{s <= t}) for the diagonal 128x128 blocks
    tri = pool.tile([128, 128], BF16, name="tri", tag="tri", bufs=1)
    bqs = pool.tile([128, 4], F32, name="bqs", tag="bias", bufs=2)
    bks = pool.tile([128, 4], F32, name="bks", tag="bias", bufs=2)
    ones_f = pool.tile([128, 64], F32, name="ones_f", tag="ones_f", bufs=1)

    # fp8 DoubleRow weights: [128, pair, s, 512] hi and lo tiles per matrix
    wqkv = {}
    for nm in ("wq", "wk", "wv"):
        wqkv[nm] = [pool.tile([128, NP, 2, CH], F8, name=f"{nm}{hl}",
                              tag="w8", bufs=6) for hl in range(2)]
    # chunk-0 x, split hi/lo for a fast start; later chunks combined
    x0 = [pool.tile([128, NP, 2, 512], F8, name=f"x0_{hl}", tag="x0", bufs=2)
          for hl in range(2)]
    xcomb = [None] * NJ  # chunks 1..3: [128, hl, pair, s, 512] tiles

    # DMA plan: sync carries wq/wk and all later x chunks; scalar carries the
    # chunk-0 x, biases, mask and wv, then stays idle so the Act engine's
    # sequencer is free once the exp stream spins up.
    nc.sync.dma_start(wqkv["wq"][0][:, 0], wq[:, 0, 0])
    nc.scalar.dma_start(x0[0][:, 0], xq[:, 0, 0, 0])
    nc.sync.dma_start(wqkv["wq"][0][:, 1:4], wq[:, 0, 1:4])
    nc.scalar.dma_start(x0[0][:, 1:4], xq[:, 0, 0, 1:4])
    nc.sync.dma_start(wqkv["wq"][1][:], wq[:, 1])
    nc.scalar.dma_start(x0[1][:], xq[:, 0, 1])
    nc.sync.dma_start(wqkv["wk"][0][:], wk[:, 0])
    nc.scalar.dma_start(wqkv["wv"][0][:], wv[:, 0])
    nc.scalar.dma_start(bqs[:], bq2[:])
    nc.scalar.dma_start(bks[:], bk2[:])
    nc.sync.dma_start(wqkv["wk"][1][:], wk[:, 1])
    nc.scalar.dma_start(tri[:], mask[:])
    nc.scalar.dma_start(wqkv["wv"][1][:], wv[:, 1])
    nc.gpsimd.memset(ones_f[:], 1.0)
    for i in range(NT):
        ocol = vp[i][:, 0:520].rearrange("p (h e) -> p h e", e=65)[:, :, 64:65]
        nc.vector.tensor_copy(ocol, ones_f[:, 0:8].unsqueeze(2))

    def load_x(j):
        xt_t = pool.tile([128, 2, NP, 2, 512], F8, name=f"xt{j}", tag="xt",
                         bufs=2)
        nc.sync.dma_start(xt_t[:], xq[:, j])
        xcomb[j] = xt_t

    def x_ap(j, p, xh, c0, cw):
        if j == 0:
            return x0[xh][:, p, :, c0:c0 + cw]
        return xcomb[j][:, xh, p, :, c0:c0 + cw]

    qtc = [[None] * NJ for _ in range(4)]   # per-chunk Q^T tiles
    otc = [[None] * NJ for _ in range(4)]   # per-chunk O^T tiles
    wps = [[None, None] for _ in range(4)]  # wp [128,512] halves, loaded late

    def _qkv_psum(idx, j, nm):
        # chunk 0 runs all four blocks of a part concurrently, term-staged,
        # so the PE can sweep the hi*hi term as soon as the hi tiles land;
        # the sp-tag PSUM slots are idle during chunk 0, borrow two of them
        if j == 0 and idx >= 2:
            return psp.tile([128, 1024], F32, name=nm, tag="sp",
                            bufs=2)[:, 0:512]
        return psp.tile([128, 512], F32, name=nm, tag="qk", bufs=2)[:]

    def emit_qkv(j, parts="qkv", sel=(0, 1, 2, 3)):
        # Q^T and K^T: out [128 qk-dims, 512 t] per m-block
        for part in parts:
            if part in "qk":
                wsrc = wqkv["wq"] if part == "q" else wqkv["wk"]
                bias_t = bqs if part == "q" else bks
                pss = [_qkv_psum(i, j, f"{part}ps{j}_{m}")
                       for i, m in enumerate(sel)]
                for ti, (xh, wh) in enumerate(TERMS):
                    for i, m in enumerate(sel):
                        for u in range(2):
                            for p in range(NP):
                                nc.tensor.matmul(
                                    pss[i][:, 256 * u:256 * u + 256],
                                    wsrc[wh][:, p, :, 128 * m:128 * m + 128],
                                    x_ap(j, p, xh, 256 * u, 256),
                                    start=(ti == 0 and p == 0),
                                    stop=(ti == NTERM - 1 and p == NP - 1),
                                    perf_mode=DRM,
                                )
                for i, m in enumerate(sel):
                    if part == "q":
                        t_ = pool.tile([128, 512], BF16, name=f"qt{m}_{j}",
                                       tag="qtc", bufs=8)
                        qtc[m][j] = t_
                        out_ap = t_[:]
                    else:
                        out_ap = kt[m][:, 512 * j:512 * j + 512]
                    nc.vector.tensor_scalar_add(out_ap, pss[i],
                                                bias_t[:, m:m + 1])
            else:
                # V: out [128 t-slice, 512 v-dims]
                pss = [_qkv_psum(i, j, f"vps{4 * j + u}")
                       for i, u in enumerate(sel)]
                for ti, (xh, wh) in enumerate(TERMS):
                    for i, u in enumerate(sel):
                        for h2 in range(2):
                            for p in range(NP):
                                nc.tensor.matmul(
                                    pss[i][:, 256 * h2:256 * h2 + 256],
                                    x_ap(j, p, xh, 128 * u, 128),
                                    wqkv["wv"][wh][:, p, :,
                                                   256 * h2:256 * h2 + 256],
                                    start=(ti == 0 and p == 0),
                                    stop=(ti == NTERM - 1 and p == NP - 1),
                                    perf_mode=DRM,
                                )
                for i, u in enumerate(sel):
                    dst = vp[4 * j + u][:, 0:520].rearrange(
                        "p (h e) -> p h e", e=65)[:, :, 0:64]
                    src = pss[i].rearrange("p (h e) -> p h e", e=64)
                    nc.vector.tensor_copy(dst, src)

    def tile_layout(p, j):
        # pairs of s-tiles per [128,1024] PSUM slot; diagonal tiles are
        # narrowed to the causally valid t-range [128r, 512).
        # entries: (i, slot_col, valid_t0, width, diag_block_col)
        i0, i1 = 2 * p, 2 * p + 1
        r0_, r1_ = i0 - 4 * j, i1 - 4 * j
        if r1_ < 0:
            return [(i0, 0, 0, 512, None), (i1, 512, 0, 512, None)], 1024
        if r0_ == 0:
            return [(i0, 0, 0, 512, 0), (i1, 512, 128, 384, 512)], 896
        return [(i0, 0, 256, 256, 0), (i1, 256, 384, 128, 256)], 384

    def emit_attn(j, mts=(0, 1, 2, 3), filler=None):
        # process the two heads of each mt pair in lockstep: the PE alternates
        # S and PV between the heads, so each head's exp runs while the other
        # head's matmul occupies the PE. `filler` emits PE work between mts
        # while the Act engine catches up on exps.
        n_i = 4 * j + 4
        npairs = n_i // 2
        for mt in mts:
            hA, hB = 2 * mt, 2 * mt + 1

            ops = {}
            ets = {}
            for h in (hA, hB):
                ops[h] = psp.tile([65, 512], F32, name=f"ops{h}_{j}", tag="o",
                                  bufs=2)

            def emit_s(h, p):
                off = 64 * (h % 2)
                layout, exp_hi = tile_layout(p, j)
                sp = psp.tile([128, 1024], F32, name=f"sp{h}_{j}_{p}", tag="sp",
                              bufs=2)
                for (i, scol, t0, w, _) in layout:
                    nc.tensor.matmul(
                        sp[:, scol:scol + w],
                        kt[mt][off:off + 64, 128 * i:128 * i + 128],
                        qtc[mt][j][off:off + 64, t0:t0 + w],
                        start=True, stop=True,
                    )
                et = pool.tile([128, 1024], BF16, name=f"et{h}_{j}_{p}",
                               tag="et", bufs=4)
                nc.scalar.activation(et[:, 0:exp_hi], sp[:, 0:exp_hi], AF.Exp,
                                     scale=0.125)
                for (i, scol, t0, w, dcol) in layout:
                    if dcol is not None:
                        blk = et[:, dcol:dcol + 128]
                        nc.gpsimd.tensor_mul(blk, blk, tri[:])
                ets[h] = (et, layout)

            def emit_pv(h, p):
                et, layout = ets[h]
                for (i, scol, t0, w, _) in layout:
                    nc.tensor.matmul(
                        ops[h][:, t0:t0 + w], vp[i][:, 65 * h:65 * h + 65],
                        et[:, scol:scol + w],
                        start=(i == 0), stop=(i == n_i - 1),
                    )

            # software pipeline across the two heads
            emit_s(hA, 0)
            for p in range(npairs):
                emit_s(hB, p)
                emit_pv(hA, p)
                if p + 1 < npairs:
                    emit_s(hA, p + 1)
                emit_pv(hB, p)

            # normalize: rows 0..63 unnormalized O^T, row 64 = Z
            # 1/Z on one partition, Pool broadcasts it across the 64 O rows
            rbs = {}
            for h in (hA, hB):
                rb1 = pool.tile([1, 512], F32R, name=f"rb1{h}_{j}", tag="rb1",
                                bufs=2)
                with nc.allow_low_precision(reason="fp32r softmax denom"):
                    nc.vector.reciprocal(rb1[:], ops[h][64:65, :])
                rbs_t = pool.tile([64, 512], F32R, name=f"rbs{h}_{j}",
                                  tag="rbs", bufs=2)
                nc.gpsimd.partition_broadcast(rbs_t[:], rb1[:])
                rbs[h] = rbs_t
            # all 16 O^T tiles stay live until their chunk's projection
            if otc[mt][j] is None:
                otc[mt][j] = pool.tile([128, 512], BF16, name=f"ot{mt}_{j}",
                                       tag="otc", bufs=16)
            nc.vector.tensor_mul(otc[mt][j][0:64, :], ops[hA][0:64, :],
                                 rbs[hA][:])
            st = pool.tile([64, 512], BF16, name=f"st{hB}_{j}", tag="st",
                           bufs=2)
            nc.vector.tensor_mul(st[:], ops[hB][0:64, :], rbs[hB][:])
            # scalar queue: idle mid-kernel, so the shift never queues behind
            # x/weight/output traffic on sync
            nc.scalar.dma_start(otc[mt][j][64:128, :], st[:])
            if filler:
                filler.pop(0)()

    def emit_wp_loads():
        for m in range(4):
            for n in range(2):
                t_ = pool.tile([128, 512], BF16, name=f"wps{m}_{n}",
                               tag="wp2", bufs=8)
                wps[m][n] = t_
                nc.sync.dma_start(
                    t_[:],
                    wp[128 * m:128 * m + 128, 512 * n:512 * n + 512],
                )

    def emit_proj(j, us=(0, 1, 2, 3), tail=False):
        # yo copies alternate DVE / Pool to spread the drain work; the
        # post-attention tail uses the freed "o" PSUM ring for double slots
        # and the now-idle Act engine for copies
        for u in us:
            t = 4 * j + u
            for n in range(2):
                tag = "o" if (tail and (u + n) % 2 == 0) else "qk"
                ps = psp.tile([128, 512], F32, name=f"yps{t}_{n}", tag=tag,
                              bufs=2)
                for m in range(4):
                    nc.tensor.matmul(
                        ps[:], otc[m][j][:, 128 * u:128 * u + 128],
                        wps[m][n][:],
                        start=(m == 0), stop=(m == 3),
                    )
                yo = pool.tile([128, 512], F32, name=f"yo{t}_{n}", tag="yo",
                               bufs=4)
                if tail:
                    nc.scalar.copy(yo[:], ps[:])
                else:
                    eng = nc.vector if (u + n) % 2 == 0 else nc.gpsimd
                    eng.tensor_copy(yo[:], ps[:])
                dma_eng = nc.scalar if (tail and (u + n) % 2 == 1) else nc.sync
                dma_eng.dma_start(
                    yout[128 * t:128 * t + 128, 512 * n:512 * n + 512], yo[:]
                )

    # ---- schedule ----
    # qkv(j+1) is woven between attn(j)'s head-pairs as PE filler while the
    # Act engine catches up on the exp backlog; attn(3) gets the projections.
    def rest(jn, *extra):
        def f():
            emit_qkv(jn, parts="q", sel=(2, 3))
            emit_qkv(jn, parts="k", sel=(2, 3))
            emit_qkv(jn, parts="v", sel=(2, 3))
            for e in extra:
                e()
        return f

    emit_qkv(0)
    load_x(1)
    emit_attn(0, filler=[lambda: emit_qkv(1, parts="q", sel=(0, 1)),
                         lambda: emit_qkv(1, parts="k", sel=(0, 1)),
                         lambda: emit_qkv(1, parts="v", sel=(0, 1)),
                         rest(1, lambda: load_x(2), emit_wp_loads)])
    emit_attn(1, filler=[lambda: emit_qkv(2, parts="q", sel=(0, 1)),
                         lambda: emit_qkv(2, parts="k", sel=(0, 1)),
                         lambda: emit_qkv(2, parts="v", sel=(0, 1)),
                         rest(2, lambda: load_x(3))])
    emit_attn(2, filler=[lambda: emit_qkv(3, parts="q", sel=(0, 1)),
                         lambda: emit_qkv(3, parts="k", sel=(0, 1)),
                         lambda: emit_qkv(3, parts="v", sel=(0, 1)),
                         rest(3)])
    emit_attn(3, filler=[lambda: emit_proj(0, us=(0, 1)),
                         lambda: emit_proj(0, us=(2, 3)),
                         lambda: emit_proj(1, us=(0, 1)),
                         lambda: None])
    emit_proj(1, us=(2, 3), tail=True)
    emit_proj(2, tail=True)
    emit_proj(3, tail=True)

    for m in range(4):
        qtc[m] = [None] * NJ
        otc[m] = [None] * NJ
    pool.release()
    psp.release()


def build(passes=1):
    key = ("nc", passes)
    if key in _CACHE:
        return _CACHE[key]
    nc = bacc.Bacc("TRN2", target_bir_lowering=False, debug=False,
                   num_devices=N_CORES)
    aps = {
        # fp8 DoubleRow operands, host-packed hl-major so chunk/pair slices
        # stay contiguous: contraction row = 256*pair + 128*s + partition,
        # hl = hi/lo residual half
        "xq": nc.dram_tensor("xq", [128, NJ, 2, NP, 2, 512], F8,
                             kind="ExternalInput").ap(),
        "wq": nc.dram_tensor("wq", [128, 2, NP, 2, CH], F8,
                             kind="ExternalInput").ap(),
        "wk": nc.dram_tensor("wk", [128, 2, NP, 2, CH], F8,
                             kind="ExternalInput").ap(),
        "wv": nc.dram_tensor("wv", [128, 2, NP, 2, CH], F8,
                             kind="ExternalInput").ap(),
        "wp": nc.dram_tensor("wp", [CH, C], BF16, kind="ExternalInput").ap(),
        "bq2": nc.dram_tensor("bq2", [128, 4], F32, kind="ExternalInput").ap(),
        "bk2": nc.dram_tensor("bk2", [128, 4], F32, kind="ExternalInput").ap(),
        "mask": nc.dram_tensor("mask", [128, 128], BF16,
                               kind="ExternalInput").ap(),
        "y": nc.dram_tensor("y", [T, C], F32, kind="ExternalOutput").ap(),
    }
    with tile.TileContext(nc) as tc:
        for _ in range(passes):
            _emit(nc, tc, aps)
    nc.compile()
    _CACHE[key] = nc
    return nc


def _pack_w(a):
    """[1024, CH] f32 -> [128, 2(hl), NP, 2(s), CH] fp8 DoubleRow pack
    with hi/lo residual split."""
    r = a.reshape(NP, 2, 128, CH)              # (pair, s, p, cols)
    hi = r.astype(FP8)
    lo = (r - hi.astype(np.float32)).astype(FP8)
    out = np.stack([hi, lo], axis=2)           # (pair, s, hl, p, cols)
    return np.ascontiguousarray(out.transpose(3, 2, 0, 1, 4))


def _pack_x(a):
    """[1024, T] f32 -> [128, NJ, 2(hl), NP, 2(s), 512] fp8 DoubleRow pack."""
    r = a.reshape(NP, 2, 128, NJ, 512)         # (pair, s, p, j, tl)
    hi = r.astype(FP8)
    lo = (r - hi.astype(np.float32)).astype(FP8)
    out = np.stack([hi, lo], axis=2)           # (pair, s, hl, p, j, tl)
    return np.ascontiguousarray(out.transpose(3, 4, 2, 0, 1, 5))


def make_in_maps(x, Wq, bq, Wk, bk, Wv, bv, Wp, bp):
    # lower-triangle 0/1 mask for the diagonal 128x128 attention blocks
    s_idx = np.arange(128)[:, None]
    t_idx = np.arange(128)[None, :]
    mask = (s_idx <= t_idx).astype(ml_dtypes.bfloat16)
    in_maps = []
    for c in range(N_CORES):
        b, g = c // 2, c % 2
        cols = slice(CH * g, CH * g + CH)
        in_maps.append({
            "xq": _pack_x(np.ascontiguousarray(x[b].T)),
            "wq": _pack_w(np.ascontiguousarray(Wq[:, cols])),
            "wk": _pack_w(np.ascontiguousarray(Wk[:, cols])),
            "wv": _pack_w(np.ascontiguousarray(Wv[:, cols])),
            "wp": np.ascontiguousarray(Wp[cols, :]).astype(ml_dtypes.bfloat16),
            "bq2": np.ascontiguousarray(bq[cols].reshape(4, 128).T),
            "bk2": np.ascontiguousarray(bk[cols].reshape(4, 128).T),
            "mask": np.ascontiguousarray(mask),
        })
    return in_maps


def kernel(x, Wq, bq, Wk, bk, Wv, bv, Wp, bp):
    # host-side prep is pure numpy; convert in case jax arrays are passed
    x, Wq, bq, Wk, bk, Wv, bv, Wp, bp = (
        np.asarray(a, dtype=np.float32)
        for a in (x, Wq, bq, Wk, bk, Wv, bv, Wp, bp)
    )
    nc = build()
    in_maps = make_in_maps(x, Wq, bq, Wk, bk, Wv, bv, Wp, bp)
    # the axon-proxied device occasionally reports a transient unrecoverable
    # exec state that clears on a fresh attempt; retry rather than fail
    last_err = None
    for _attempt in range(3):
        try:
            res = run_bass_kernel_spmd(nc, in_maps, core_ids=list(range(N_CORES)))
            break
        except Exception as e:  # noqa: BLE001
            last_err = e
            import time as _time
            _time.sleep(5)
    else:
        raise last_err
    corr = (bv @ Wp + bp).astype(np.float32)
    out = np.empty((B, T, C), dtype=np.float32)
    for b in range(B):
        out[b] = res.results[2 * b]["y"] + res.results[2 * b + 1]["y"] + corr
    return out


# revision 26
# speedup vs baseline: 1.0325x; 1.0050x over previous
"""Causal self-attention (B=4, T=2048, C=1024, H=16, D=64) on 8 trn2 cores.

Sharding: data-parallel over B (4) x tensor-parallel over head-halves (2).
Core c handles batch c//2 with heads [8*(c%2), 8*(c%2)+8). Each core emits a
partial projection output [2048, 1024]; host sums the two head-half partials
per batch and adds the (bv @ Wp + bp) correction row.

Device layout highlights:
 - QKV projections run as fp8e4m3 DoubleRow matmuls (2 k-tiles per pass, 0.5
   cycles/row) with a 3-term residual split prepared on the host:
   x@w ~= x_hi@w_hi + x_lo@w_hi + x_hi@w_lo, each operand quantized e4m3.
   This is 3/4 the PE cost of fp32r at ~5e-3 max rel err per GEMM.
 - everything downstream of the QKV psums is bf16: Q^T/K^T/V tiles, exp(S)
   tiles, O^T tiles and Wp. bf16 matmuls run at 1.0 cycles/row at ANY width
   (fp32r pays 4x below 256), halve SBUF footprint and DMA bytes, and cost
   ~0.3% relative error against a 2e-2 budget.
 - S^T = K^T.T @ Q^T keeps softmax denominators computable by an in-matmul
   ones-column (V' has a 65th column of ones -> row 64 of O' = Z)
 - softmax skips max-subtraction (logits are ~N(0,1); exp cannot overflow)
 - causal masking via 0/1 mask multiply on the diagonal-block patterns,
   executed on the Pool engine (otherwise idle)
 - softmax denominators: DVE reciprocal of the Z row + Pool-engine
   partition_broadcast (replaces the former ones-column PE matmul)
 - attention processes two heads in lockstep so the PE never waits on the
   Act engine's exp: S_a, S_b, PV_a, S_a', PV_b, ...; QKV/projection work is
   woven between head-pairs as PE filler while Act drains exps
 - projection runs as 8 K=64 groups against per-head-half [64,512] O^T tiles,
   so both heads' normalize muls write partition-0-based tiles and the old
   partition-shifting SBUF->SBUF DMA disappears
"""

import os
import sys

for _p in ("/opt/trn_rl_repo", "/root/.axon_site/_ro/trn_rl_repo"):
    if os.path.isdir(_p) and _p not in sys.path:
        sys.path.insert(0, _p)

import ml_dtypes
import numpy as np
from concourse import bacc, mybir, tile
from concourse.bass_utils import run_bass_kernel_spmd

N_CORES = 8
B, T, C = 4, 2048, 1024
H, D = 16, 64          # full model heads
HG = 8                 # heads per core (head-group)
CH = HG * D            # 512, per-core qkv width
NT = T // 128          # 16 s-tiles
NJ = T // 512          # 4 t-chunks
NP = C // 256          # 4 DoubleRow contraction pairs
F32 = mybir.dt.float32
F32R = mybir.dt.float32r
BF16 = mybir.dt.bfloat16
F8 = mybir.dt.float8e4
DRM = mybir.MatmulPerfMode.DoubleRow
AF = mybir.ActivationFunctionType
FP8 = ml_dtypes.float8_e4m3

# (x_half, w_half) residual terms; term-major so the hi*hi sweep only needs
# the hi tiles that arrive first, then x-lo (scalar queue) before w-lo (sync)
TERMS = ((0, 0), (1, 0), (0, 1))
NTERM = len(TERMS)

_CACHE = {}


def _emit(nc, tc, aps):
    xq, wq, wk, wv, wp, bq2, bk2, mask, yout = (
        aps["xq"], aps["wq"], aps["wk"], aps["wv"], aps["wp"],
        aps["bq2"], aps["bk2"], aps["mask"], aps["y"],
    )

    pool = tc.alloc_tile_pool(name="pool", bufs=1)
    psp = tc.alloc_tile_pool(name="ps", bufs=1, space="PSUM")

    # ---- persistent tensors ----
    kt = [pool.tile([128, T], BF16, name=f"kt{m}", tag="kt", bufs=4)
          for m in range(4)]
    vp = [pool.tile([128, 520], BF16, name=f"vp{i}", tag="vp", bufs=NT)
          for i in range(NT)]
    # single lower-triangle mask (1{s <= t}) for the diagonal 128x128 blocks
    tri = pool.tile([128, 128], BF16, name="tri", tag="tri", bufs=1)
    bqs = pool.tile([128, 4], F32, name="bqs", tag="bias", bufs=2)
    bks = pool.tile([128, 4], F32, name="bks", tag="bias", bufs=2)
    ones_f = pool.tile([128, 64], F32, name="ones_f", tag="ones_f", bufs=1)

    # fp8 DoubleRow weights: [128, pair, s, 512] hi and lo tiles per matrix
    wqkv = {}
    for nm in ("wq", "wk", "wv"):
        wqkv[nm] = [pool.tile([128, NP, 2, CH], F8, name=f"{nm}{hl}",
                              tag="w8", bufs=6) for hl in range(2)]
    # chunk-0 x, split hi/lo for a fast start; later chunks combined
    x0 = [pool.tile([128, NP, 2, 512], F8, name=f"x0_{hl}", tag="x0", bufs=2)
          for hl in range(2)]
    xcomb = [None] * NJ  # chunks 1..3: [128, hl, pair, s, 512] tiles

    # DMA plan: sync carries wq/wk and all later x chunks; scalar carries the
    # chunk-0 x, biases, mask and wv, then stays idle so the Act engine's
    # sequencer is free once the exp stream spins up.
    nc.sync.dma_start(wqkv["wq"][0][:, 0], wq[:, 0, 0])
    nc.scalar.dma_start(x0[0][:, 0], xq[:, 0, 0, 0])
    nc.sync.dma_start(wqkv["wq"][0][:, 1:4], wq[:, 0, 1:4])
    nc.scalar.dma_start(x0[0][:, 1:4], xq[:, 0, 0, 1:4])
    nc.sync.dma_start(wqkv["wq"][1][:], wq[:, 1])
    nc.scalar.dma_start(x0[1][:], xq[:, 0, 1])
    nc.sync.dma_start(wqkv["wk"][0][:], wk[:, 0])
    nc.scalar.dma_start(wqkv["wv"][0][:], wv[:, 0])
    nc.scalar.dma_start(bqs[:], bq2[:])
    nc.scalar.dma_start(bks[:], bk2[:])
    nc.sync.dma_start(wqkv["wk"][1][:], wk[:, 1])
    nc.scalar.dma_start(tri[:], mask[:])
    nc.scalar.dma_start(wqkv["wv"][1][:], wv[:, 1])
    nc.gpsimd.memset(ones_f[:], 1.0)
    for i in range(NT):
        ocol = vp[i][:, 0:520].rearrange("p (h e) -> p h e", e=65)[:, :, 64:65]
        nc.vector.tensor_copy(ocol, ones_f[:, 0:8].unsqueeze(2))

    def load_x(j):
        xt_t = pool.tile([128, 2, NP, 2, 512], F8, name=f"xt{j}", tag="xt",
                         bufs=2)
        nc.sync.dma_start(xt_t[:], xq[:, j])
        xcomb[j] = xt_t

    def x_ap(j, p, xh, c0, cw):
        if j == 0:
            return x0[xh][:, p, :, c0:c0 + cw]
        return xcomb[j][:, xh, p, :, c0:c0 + cw]

    qtc = [[None] * NJ for _ in range(4)]   # per-chunk Q^T tiles
    otc = [[None] * NJ for _ in range(4)]   # per-chunk O^T tiles
    wps = [[None, None] for _ in range(4)]  # wp [128,512] halves, loaded late

    def _qkv_psum(idx, j, nm):
        # chunk 0 runs all four blocks of a part concurrently, term-staged,
        # so the PE can sweep the hi*hi term as soon as the hi tiles land;
        # the sp-tag PSUM slots are idle during chunk 0, borrow two of them
        if j == 0 and idx >= 2:
            return psp.tile([128, 1024], F32, name=nm, tag="sp",
                            bufs=2)[:, 0:512]
        return psp.tile([128, 512], F32, name=nm, tag="qk", bufs=2)[:]

    def emit_qkv(j, parts="qkv", sel=(0, 1, 2, 3)):
        # Q^T and K^T: out [128 qk-dims, 512 t] per m-block
        for part in parts:
            if part in "qk":
                wsrc = wqkv["wq"] if part == "q" else wqkv["wk"]
                bias_t = bqs if part == "q" else bks
                pss = [_qkv_psum(i, j, f"{part}ps{j}_{m}")
                       for i, m in enumerate(sel)]
                for ti, (xh, wh) in enumerate(TERMS):
                    for i, m in enumerate(sel):
                        for u in range(2):
                            for p in range(NP):
                                nc.tensor.matmul(
                                    pss[i][:, 256 * u:256 * u + 256],
                                    wsrc[wh][:, p, :, 128 * m:128 * m + 128],
                                    x_ap(j, p, xh, 256 * u, 256),
                                    start=(ti == 0 and p == 0),
                                    stop=(ti == NTERM - 1 and p == NP - 1),
                                    perf_mode=DRM,
                                )
                for i, m in enumerate(sel):
                    if part == "q":
                        t_ = pool.tile([128, 512], BF16, name=f"qt{m}_{j}",
                                       tag="qtc", bufs=8)
                        qtc[m][j] = t_
                        out_ap = t_[:]
                    else:
                        out_ap = kt[m][:, 512 * j:512 * j + 512]
                    nc.vector.tensor_scalar_add(out_ap, pss[i],
                                                bias_t[:, m:m + 1])
            else:
                # V: out [128 t-slice, 512 v-dims]
                pss = [_qkv_psum(i, j, f"vps{4 * j + u}")
                       for i, u in enumerate(sel)]
                for ti, (xh, wh) in enumerate(TERMS):
                    for i, u in enumerate(sel):
                        for h2 in range(2):
                            for p in range(NP):
                                nc.tensor.matmul(
                                    pss[i][:, 256 * h2:256 * h2 + 256],
                                    x_ap(j, p, xh, 128 * u, 128),
                                    wqkv["wv"][wh][:, p, :,
                                                   256 * h2:256 * h2 + 256],
                                    start=(ti == 0 and p == 0),
                                    stop=(ti == NTERM - 1 and p == NP - 1),
                                    perf_mode=DRM,
                                )
                for i, u in enumerate(sel):
                    dst = vp[4 * j + u][:, 0:520].rearrange(
                        "p (h e) -> p h e", e=65)[:, :, 0:64]
                    src = pss[i].rearrange("p (h e) -> p h e", e=64)
                    nc.vector.tensor_copy(dst, src)

    def tile_layout(p, j):
        # pairs of s-tiles per [128,1024] PSUM slot; diagonal tiles are
        # narrowed to the causally valid t-range [128r, 512).
        # entries: (i, slot_col, valid_t0, width, diag_block_col)
        i0, i1 = 2 * p, 2 * p + 1
        r0_, r1_ = i0 - 4 * j, i1 - 4 * j
        if r1_ < 0:
            return [(i0, 0, 0, 512, None), (i1, 512, 0, 512, None)], 1024
        if r0_ == 0:
            return [(i0, 0, 0, 512, 0), (i1, 512, 128, 384, 512)], 896
        return [(i0, 0, 256, 256, 0), (i1, 256, 384, 128, 256)], 384

    def emit_attn(j, mts=(0, 1, 2, 3), filler=None):
        # process the two heads of each mt pair in lockstep: the PE alternates
        # S and PV between the heads, so each head's exp runs while the other
        # head's matmul occupies the PE. `filler` emits PE work between mts
        # while the Act engine catches up on exps.
        n_i = 4 * j + 4
        npairs = n_i // 2
        for mt in mts:
            hA, hB = 2 * mt, 2 * mt + 1

            ops = {}
            ets = {}
            for h in (hA, hB):
                ops[h] = psp.tile([65, 512], F32, name=f"ops{h}_{j}", tag="o",
                                  bufs=2)

            def emit_s(h, p):
                off = 64 * (h % 2)
                layout, exp_hi = tile_layout(p, j)
                sp = psp.tile([128, 1024], F32, name=f"sp{h}_{j}_{p}", tag="sp",
                              bufs=2)
                for (i, scol, t0, w, _) in layout:
                    nc.tensor.matmul(
                        sp[:, scol:scol + w],
                        kt[mt][off:off + 64, 128 * i:128 * i + 128],
                        qtc[mt][j][off:off + 64, t0:t0 + w],
                        start=True, stop=True,
                    )
                et = pool.tile([128, 1024], BF16, name=f"et{h}_{j}_{p}",
                               tag="et", bufs=4)
                nc.scalar.activation(et[:, 0:exp_hi], sp[:, 0:exp_hi], AF.Exp,
                                     scale=0.125)
                for (i, scol, t0, w, dcol) in layout:
                    if dcol is not None:
                        blk = et[:, dcol:dcol + 128]
                        nc.gpsimd.tensor_mul(blk, blk, tri[:])
                ets[h] = (et, layout)

            def emit_pv(h, p):
                et, layout = ets[h]
                for (i, scol, t0, w, _) in layout:
                    nc.tensor.matmul(
                        ops[h][:, t0:t0 + w], vp[i][:, 65 * h:65 * h + 65],
                        et[:, scol:scol + w],
                        start=(i == 0), stop=(i == n_i - 1),
                    )

            # software pipeline across the two heads
            emit_s(hA, 0)
            for p in range(npairs):
                emit_s(hB, p)
                emit_pv(hA, p)
                if p + 1 < npairs:
                    emit_s(hA, p + 1)
                emit_pv(hB, p)

            # normalize: rows 0..63 unnormalized O^T, row 64 = Z
            # 1/Z on one partition, Pool broadcasts it across the 64 O rows
            rbs = {}
            for h in (hA, hB):
                rb1 = pool.tile([1, 512], F32R, name=f"rb1{h}_{j}", tag="rb1",
                                bufs=2)
                with nc.allow_low_precision(reason="fp32r softmax denom"):
                    nc.vector.reciprocal(rb1[:], ops[h][64:65, :])
                rbs_t = pool.tile([64, 512], F32R, name=f"rbs{h}_{j}",
                                  tag="rbs", bufs=2)
                nc.gpsimd.partition_broadcast(rbs_t[:], rb1[:])
                rbs[h] = rbs_t
            # all 16 O^T tiles stay live until their chunk's projection
            if otc[mt][j] is None:
                otc[mt][j] = pool.tile([128, 512], BF16, name=f"ot{mt}_{j}",
                                       tag="otc", bufs=16)
            nc.vector.tensor_mul(otc[mt][j][0:64, :], ops[hA][0:64, :],
                                 rbs[hA][:])
            st = pool.tile([64, 512], BF16, name=f"st{hB}_{j}", tag="st",
                           bufs=2)
            nc.vector.tensor_mul(st[:], ops[hB][0:64, :], rbs[hB][:])
            # Pool SWDGE queue: keeps the shift off both HWDGE queues and
            # the Act sequencer
            nc.gpsimd.dma_start(otc[mt][j][64:128, :], st[:])
            if filler:
                filler.pop(0)()

    def emit_wp_loads():
        for m in range(4):
            for n in range(2):
                t_ = pool.tile([128, 512], BF16, name=f"wps{m}_{n}",
                               tag="wp2", bufs=8)
                wps[m][n] = t_
                nc.sync.dma_start(
                    t_[:],
                    wp[128 * m:128 * m + 128, 512 * n:512 * n + 512],
                )

    def emit_proj(j, us=(0, 1, 2, 3), tail=False):
        # yo copies alternate DVE / Pool to spread the drain work; the
        # post-attention tail uses the freed "o" PSUM ring for double slots
        # and the now-idle Act engine for copies
        for u in us:
            t = 4 * j + u
            for n in range(2):
                tag = "o" if (tail and (u + n) % 2 == 0) else "qk"
                ps = psp.tile([128, 512], F32, name=f"yps{t}_{n}", tag=tag,
                              bufs=2)
                for m in range(4):
                    nc.tensor.matmul(
                        ps[:], otc[m][j][:, 128 * u:128 * u + 128],
                        wps[m][n][:],
                        start=(m == 0), stop=(m == 3),
                    )
                yo = pool.tile([128, 512], F32, name=f"yo{t}_{n}", tag="yo",
                               bufs=4)
                eng = nc.vector if (u + n) % 2 == 0 else nc.gpsimd
                eng.tensor_copy(yo[:], ps[:])
                dma_eng = nc.scalar if (tail and (u + n) % 2 == 1) else nc.sync
                dma_eng.dma_start(
                    yout[128 * t:128 * t + 128, 512 * n:512 * n + 512], yo[:]
                )

    # ---- schedule ----
    # qkv(j+1) is woven between attn(j)'s head-pairs as PE filler while the
    # Act engine catches up on the exp backlog; attn(3) gets the projections.
    def rest(jn, *extra):
        def f():
            emit_qkv(jn, parts="q", sel=(2, 3))
            emit_qkv(jn, parts="k", sel=(2, 3))
            emit_qkv(jn, parts="v", sel=(2, 3))
            for e in extra:
                e()
        return f

    emit_qkv(0)
    load_x(1)
    emit_attn(0, filler=[lambda: emit_qkv(1, parts="q", sel=(0, 1)),
                         lambda: emit_qkv(1, parts="k", sel=(0, 1)),
                         lambda: emit_qkv(1, parts="v", sel=(0, 1)),
                         rest(1, lambda: load_x(2), emit_wp_loads)])
    emit_attn(1, filler=[lambda: emit_qkv(2, parts="q", sel=(0, 1)),
                         lambda: emit_qkv(2, parts="k", sel=(0, 1)),
                         lambda: emit_qkv(2, parts="v", sel=(0, 1)),
                         rest(2, lambda: load_x(3))])
    emit_attn(2, filler=[lambda: emit_qkv(3, parts="q", sel=(0, 1)),
                         lambda: emit_qkv(3, parts="k", sel=(0, 1)),
                         lambda: emit_qkv(3, parts="v", sel=(0, 1)),
                         rest(3)])
    emit_attn(3, filler=[lambda: emit_proj(0, us=(0, 1)),
                         lambda: emit_proj(0, us=(2, 3)),
                         lambda: emit_proj(1, us=(0, 1)),
                         lambda: None])
    emit_proj(1, us=(2, 3), tail=True)
    emit_proj(2, tail=True)
    emit_proj(3, tail=True)

    for m in range(4):
        qtc[m] = [None] * NJ
        otc[m] = [None] * NJ
    pool.release()
    psp.release()


def build(passes=1):
    key = ("nc", passes)
    if key in _CACHE:
        return _CACHE[key]
    nc = bacc.Bacc("TRN2", target_bir_lowering=False, debug=False,
                   num_devices=N_CORES)
    aps = {
        # fp8 DoubleRow operands, host-packed hl-major so chunk/pair slices
        # stay contiguous: contraction row = 256*pair + 128*s + partition,
        # hl = hi/lo residual half
        "xq": nc.dram_tensor("xq", [128, NJ, 2, NP, 2, 512], F8,
                             kind="ExternalInput").ap(),
        "wq": nc.dram_tensor("wq", [128, 2, NP, 2, CH], F8,
                             kind="ExternalInput").ap(),
        "wk": nc.dram_tensor("wk", [128, 2, NP, 2, CH], F8,
                             kind="ExternalInput").ap(),
        "wv": nc.dram_tensor("wv", [128, 2, NP, 2, CH], F8,
                             kind="ExternalInput").ap(),
        "wp": nc.dram_tensor("wp", [CH, C], BF16, kind="ExternalInput").ap(),
        "bq2": nc.dram_tensor("bq2", [128, 4], F32, kind="ExternalInput").ap(),
        "bk2": nc.dram_tensor("bk2", [128, 4], F32, kind="ExternalInput").ap(),
        "mask": nc.dram_tensor("mask", [128, 128], BF16,
                               kind="ExternalInput").ap(),
        "y": nc.dram_tensor("y", [T, C], F32, kind="ExternalOutput").ap(),
    }
    with tile.TileContext(nc) as tc:
        for _ in range(passes):
            _emit(nc, tc, aps)
    nc.compile()
    _CACHE[key] = nc
    return nc


def _pack_w(a):
    """[1024, CH] f32 -> [128, 2(hl), NP, 2(s), CH] fp8 DoubleRow pack
    with hi/lo residual split."""
    r = a.reshape(NP, 2, 128, CH)              # (pair, s, p, cols)
    hi = r.astype(FP8)
    lo = (r - hi.astype(np.float32)).astype(FP8)
    out = np.stack([hi, lo], axis=2)           # (pair, s, hl, p, cols)
    return np.ascontiguousarray(out.transpose(3, 2, 0, 1, 4))


def _pack_x(a):
    """[1024, T] f32 -> [128, NJ, 2(hl), NP, 2(s), 512] fp8 DoubleRow pack."""
    r = a.reshape(NP, 2, 128, NJ, 512)         # (pair, s, p, j, tl)
    hi = r.astype(FP8)
    lo = (r - hi.astype(np.float32)).astype(FP8)
    out = np.stack([hi, lo], axis=2)           # (pair, s, hl, p, j, tl)
    return np.ascontiguousarray(out.transpose(3, 4, 2, 0, 1, 5))


def make_in_maps(x, Wq, bq, Wk, bk, Wv, bv, Wp, bp):
    # lower-triangle 0/1 mask for the diagonal 128x128 attention blocks
    s_idx = np.arange(128)[:, None]
    t_idx = np.arange(128)[None, :]
    mask = (s_idx <= t_idx).astype(ml_dtypes.bfloat16)
    in_maps = []
    for c in range(N_CORES):
        b, g = c // 2, c % 2
        cols = slice(CH * g, CH * g + CH)
        in_maps.append({
            "xq": _pack_x(np.ascontiguousarray(x[b].T)),
            "wq": _pack_w(np.ascontiguousarray(Wq[:, cols])),
            "wk": _pack_w(np.ascontiguousarray(Wk[:, cols])),
            "wv": _pack_w(np.ascontiguousarray(Wv[:, cols])),
            "wp": np.ascontiguousarray(Wp[cols, :]).astype(ml_dtypes.bfloat16),
            "bq2": np.ascontiguousarray(bq[cols].reshape(4, 128).T),
            "bk2": np.ascontiguousarray(bk[cols].reshape(4, 128).T),
            "mask": np.ascontiguousarray(mask),
        })
    return in_maps


def kernel(x, Wq, bq, Wk, bk, Wv, bv, Wp, bp):
    # host-side prep is pure numpy; convert in case jax arrays are passed
    x, Wq, bq, Wk, bk, Wv, bv, Wp, bp = (
        np.asarray(a, dtype=np.float32)
        for a in (x, Wq, bq, Wk, bk, Wv, bv, Wp, bp)
    )
    nc = build()
    in_maps = make_in_maps(x, Wq, bq, Wk, bk, Wv, bv, Wp, bp)
    # the axon-proxied device occasionally reports a transient unrecoverable
    # exec state that clears on a fresh attempt; retry rather than fail
    last_err = None
    for _attempt in range(3):
        try:
            res = run_bass_kernel_spmd(nc, in_maps, core_ids=list(range(N_CORES)))
            break
        except Exception as e:  # noqa: BLE001
            last_err = e
            import time as _time
            _time.sleep(5)
    else:
        raise last_err
    corr = (bv @ Wp + bp).astype(np.float32)
    out = np.empty((B, T, C), dtype=np.float32)
    for b in range(B):
        out[b] = res.results[2 * b]["y"] + res.results[2 * b + 1]["y"] + corr
    return out


# revision 27
# speedup vs baseline: 1.0528x; 1.0196x over previous
"""Causal self-attention (B=4, T=2048, C=1024, H=16, D=64) on 8 trn2 cores.

Sharding: data-parallel over B (4) x tensor-parallel over head-halves (2).
Core c handles batch c//2 with heads [8*(c%2), 8*(c%2)+8). Each core emits a
partial projection output [2048, 1024]; host sums the two head-half partials
per batch and adds the (bv @ Wp + bp) correction row.

Device layout highlights:
 - QKV projections run as fp8e4m3 DoubleRow matmuls (2 k-tiles per pass, 0.5
   cycles/row) with a 3-term residual split prepared on the host:
   x@w ~= x_hi@w_hi + x_lo@w_hi + x_hi@w_lo, each operand quantized e4m3.
   This is 3/4 the PE cost of fp32r at ~5e-3 max rel err per GEMM.
 - everything downstream of the QKV psums is bf16: Q^T/K^T/V tiles, exp(S)
   tiles, O^T tiles and Wp. bf16 matmuls run at 1.0 cycles/row at ANY width
   (fp32r pays 4x below 256), halve SBUF footprint and DMA bytes, and cost
   ~0.3% relative error against a 2e-2 budget.
 - S^T = K^T.T @ Q^T keeps softmax denominators computable by an in-matmul
   ones-column (V' has a 65th column of ones -> row 64 of O' = Z)
 - softmax skips max-subtraction (logits are ~N(0,1); exp cannot overflow)
 - causal masking via 0/1 mask multiply on the diagonal-block patterns,
   executed on the Pool engine (otherwise idle)
 - softmax denominators: DVE reciprocal of the Z row + Pool-engine
   partition_broadcast (replaces the former ones-column PE matmul)
 - attention processes two heads in lockstep so the PE never waits on the
   Act engine's exp: S_a, S_b, PV_a, S_a', PV_b, ...; QKV/projection work is
   woven between head-pairs as PE filler while Act drains exps
 - projection runs as 8 K=64 groups against per-head-half [64,512] O^T tiles,
   so both heads' normalize muls write partition-0-based tiles and the old
   partition-shifting SBUF->SBUF DMA disappears
"""

import os
import sys

for _p in ("/opt/trn_rl_repo", "/root/.axon_site/_ro/trn_rl_repo"):
    if os.path.isdir(_p) and _p not in sys.path:
        sys.path.insert(0, _p)

import ml_dtypes
import numpy as np
from concourse import bacc, mybir, tile
from concourse.bass_utils import run_bass_kernel_spmd

N_CORES = 8
B, T, C = 4, 2048, 1024
H, D = 16, 64          # full model heads
HG = 8                 # heads per core (head-group)
CH = HG * D            # 512, per-core qkv width
NT = T // 128          # 16 s-tiles
NJ = T // 512          # 4 t-chunks
NP = C // 256          # 4 DoubleRow contraction pairs
F32 = mybir.dt.float32
F32R = mybir.dt.float32r
BF16 = mybir.dt.bfloat16
F8 = mybir.dt.float8e4
DRM = mybir.MatmulPerfMode.DoubleRow
AF = mybir.ActivationFunctionType
FP8 = ml_dtypes.float8_e4m3

# (x_half, w_half) residual terms; term-major so the hi*hi sweep only needs
# the hi tiles that arrive first, then x-lo (scalar queue) before w-lo (sync)
TERMS = ((0, 0), (1, 0), (0, 1))
NTERM = len(TERMS)

_CACHE = {}


def _emit(nc, tc, aps):
    xq, wq, wk, wv, wp, bq2, bk2, mask, yout = (
        aps["xq"], aps["wq"], aps["wk"], aps["wv"], aps["wp"],
        aps["bq2"], aps["bk2"], aps["mask"], aps["y"],
    )

    pool = tc.alloc_tile_pool(name="pool", bufs=1)
    psp = tc.alloc_tile_pool(name="ps", bufs=1, space="PSUM")

    # ---- persistent tensors ----
    kt = [pool.tile([128, T], BF16, name=f"kt{m}", tag="kt", bufs=4)
          for m in range(4)]
    vp = [pool.tile([128, 520], BF16, name=f"vp{i}", tag="vp", bufs=NT)
          for i in range(NT)]
    # single lower-triangle mask (1{s <= t}) for the diagonal 128x128 blocks
    tri = pool.tile([128, 128], BF16, name="tri", tag="tri", bufs=1)
    bqs = pool.tile([128, 4], F32, name="bqs", tag="bias", bufs=2)
    bks = pool.tile([128, 4], F32, name="bks", tag="bias", bufs=2)
    ones_f = pool.tile([128, 64], F32, name="ones_f", tag="ones_f", bufs=1)

    # fp8 DoubleRow weights: [128, pair, s, 512] hi and lo tiles per matrix
    wqkv = {}
    for nm in ("wq", "wk", "wv"):
        wqkv[nm] = [pool.tile([128, NP, 2, CH], F8, name=f"{nm}{hl}",
                              tag="w8", bufs=6) for hl in range(2)]
    # chunk-0 x, split hi/lo for a fast start; later chunks combined
    x0 = [pool.tile([128, NP, 2, 512], F8, name=f"x0_{hl}", tag="x0", bufs=2)
          for hl in range(2)]
    xcomb = [None] * NJ  # chunks 1..3: [128, hl, pair, s, 512] tiles

    # DMA plan: sync carries wq/wk and all later x chunks; scalar carries the
    # chunk-0 x, biases, mask and wv, then stays idle so the Act engine's
    # sequencer is free once the exp stream spins up.
    nc.sync.dma_start(wqkv["wq"][0][:, 0], wq[:, 0, 0])
    nc.scalar.dma_start(x0[0][:, 0], xq[:, 0, 0, 0])
    nc.sync.dma_start(wqkv["wq"][0][:, 1:4], wq[:, 0, 1:4])
    nc.scalar.dma_start(x0[0][:, 1:4], xq[:, 0, 0, 1:4])
    nc.sync.dma_start(wqkv["wq"][1][:], wq[:, 1])
    nc.scalar.dma_start(x0[1][:], xq[:, 0, 1])
    nc.sync.dma_start(wqkv["wk"][0][:], wk[:, 0])
    nc.scalar.dma_start(wqkv["wv"][0][:], wv[:, 0])
    nc.scalar.dma_start(bqs[:], bq2[:])
    nc.scalar.dma_start(bks[:], bk2[:])
    nc.sync.dma_start(wqkv["wk"][1][:], wk[:, 1])
    nc.scalar.dma_start(tri[:], mask[:])
    nc.scalar.dma_start(wqkv["wv"][1][:], wv[:, 1])
    nc.gpsimd.memset(ones_f[:], 1.0)
    for i in range(NT):
        ocol = vp[i][:, 0:520].rearrange("p (h e) -> p h e", e=65)[:, :, 64:65]
        nc.vector.tensor_copy(ocol, ones_f[:, 0:8].unsqueeze(2))

    def load_x(j):
        xt_t = pool.tile([128, 2, NP, 2, 512], F8, name=f"xt{j}", tag="xt",
                         bufs=2)
        nc.sync.dma_start(xt_t[:], xq[:, j])
        xcomb[j] = xt_t

    def x_ap(j, p, xh, c0, cw):
        if j == 0:
            return x0[xh][:, p, :, c0:c0 + cw]
        return xcomb[j][:, xh, p, :, c0:c0 + cw]

    qtc = [[None] * NJ for _ in range(4)]   # per-chunk Q^T tiles
    otc = [[None] * NJ for _ in range(4)]   # per-chunk O^T tiles
    wps = [[None, None] for _ in range(4)]  # wp [128,512] halves, loaded late

    def _qkv_psum(idx, j, nm):
        # chunk 0 runs all four blocks of a part concurrently, term-staged,
        # so the PE can sweep the hi*hi term as soon as the hi tiles land;
        # the sp-tag PSUM slots are idle during chunk 0, borrow two of them
        if j == 0 and idx >= 2:
            return psp.tile([128, 1024], F32, name=nm, tag="sp",
                            bufs=2)[:, 0:512]
        return psp.tile([128, 512], F32, name=nm, tag="qk", bufs=2)[:]

    def emit_qkv(j, parts="qkv", sel=(0, 1, 2, 3)):
        # Q^T and K^T: out [128 qk-dims, 512 t] per m-block
        for part in parts:
            if part in "qk":
                wsrc = wqkv["wq"] if part == "q" else wqkv["wk"]
                bias_t = bqs if part == "q" else bks
                pss = [_qkv_psum(i, j, f"{part}ps{j}_{m}")
                       for i, m in enumerate(sel)]
                for ti, (xh, wh) in enumerate(TERMS):
                    for i, m in enumerate(sel):
                        for u in range(2):
                            for p in range(NP):
                                nc.tensor.matmul(
                                    pss[i][:, 256 * u:256 * u + 256],
                                    wsrc[wh][:, p, :, 128 * m:128 * m + 128],
                                    x_ap(j, p, xh, 256 * u, 256),
                                    start=(ti == 0 and p == 0),
                                    stop=(ti == NTERM - 1 and p == NP - 1),
                                    perf_mode=DRM,
                                )
                for i, m in enumerate(sel):
                    if part == "q":
                        t_ = pool.tile([128, 512], BF16, name=f"qt{m}_{j}",
                                       tag="qtc", bufs=8)
                        qtc[m][j] = t_
                        out_ap = t_[:]
                    else:
                        out_ap = kt[m][:, 512 * j:512 * j + 512]
                    nc.vector.tensor_scalar_add(out_ap, pss[i],
                                                bias_t[:, m:m + 1])
            else:
                # V: out [128 t-slice, 512 v-dims]
                pss = [_qkv_psum(i, j, f"vps{4 * j + u}")
                       for i, u in enumerate(sel)]
                for ti, (xh, wh) in enumerate(TERMS):
                    for i, u in enumerate(sel):
                        for h2 in range(2):
                            for p in range(NP):
                                nc.tensor.matmul(
                                    pss[i][:, 256 * h2:256 * h2 + 256],
                                    x_ap(j, p, xh, 128 * u, 128),
                                    wqkv["wv"][wh][:, p, :,
                                                   256 * h2:256 * h2 + 256],
                                    start=(ti == 0 and p == 0),
                                    stop=(ti == NTERM - 1 and p == NP - 1),
                                    perf_mode=DRM,
                                )
                for i, u in enumerate(sel):
                    dst = vp[4 * j + u][:, 0:520].rearrange(
                        "p (h e) -> p h e", e=65)[:, :, 0:64]
                    src = pss[i].rearrange("p (h e) -> p h e", e=64)
                    nc.vector.tensor_copy(dst, src)

    def tile_layout(p, j):
        # pairs of s-tiles per [128,1024] PSUM slot; diagonal tiles are
        # narrowed to the causally valid t-range [128r, 512).
        # entries: (i, slot_col, valid_t0, width, diag_block_col)
        i0, i1 = 2 * p, 2 * p + 1
        r0_, r1_ = i0 - 4 * j, i1 - 4 * j
        if r1_ < 0:
            return [(i0, 0, 0, 512, None), (i1, 512, 0, 512, None)], 1024
        if r0_ == 0:
            return [(i0, 0, 0, 512, 0), (i1, 512, 128, 384, 512)], 896
        return [(i0, 0, 256, 256, 0), (i1, 256, 384, 128, 256)], 384

    def emit_attn(j, mts=(0, 1, 2, 3), filler=None):
        # process the two heads of each mt pair in lockstep: the PE alternates
        # S and PV between the heads, so each head's exp runs while the other
        # head's matmul occupies the PE. `filler` emits PE work between mts
        # while the Act engine catches up on exps.
        n_i = 4 * j + 4
        npairs = n_i // 2
        for mt in mts:
            hA, hB = 2 * mt, 2 * mt + 1

            ops = {}
            ets = {}
            for h in (hA, hB):
                ops[h] = psp.tile([65, 512], F32, name=f"ops{h}_{j}", tag="o",
                                  bufs=2)

            def emit_s(h, p):
                off = 64 * (h % 2)
                layout, exp_hi = tile_layout(p, j)
                sp = psp.tile([128, 1024], F32, name=f"sp{h}_{j}_{p}", tag="sp",
                              bufs=2)
                for (i, scol, t0, w, _) in layout:
                    nc.tensor.matmul(
                        sp[:, scol:scol + w],
                        kt[mt][off:off + 64, 128 * i:128 * i + 128],
                        qtc[mt][j][off:off + 64, t0:t0 + w],
                        start=True, stop=True,
                    )
                et = pool.tile([128, 1024], BF16, name=f"et{h}_{j}_{p}",
                               tag="et", bufs=4)
                nc.scalar.activation(et[:, 0:exp_hi], sp[:, 0:exp_hi], AF.Exp,
                                     scale=0.125)
                for (i, scol, t0, w, dcol) in layout:
                    if dcol is not None:
                        blk = et[:, dcol:dcol + 128]
                        nc.gpsimd.tensor_mul(blk, blk, tri[:])
                ets[h] = (et, layout)

            def emit_pv(h, p):
                et, layout = ets[h]
                for (i, scol, t0, w, _) in layout:
                    nc.tensor.matmul(
                        ops[h][:, t0:t0 + w], vp[i][:, 65 * h:65 * h + 65],
                        et[:, scol:scol + w],
                        start=(i == 0), stop=(i == n_i - 1),
                    )

            # software pipeline across the two heads
            emit_s(hA, 0)
            for p in range(npairs):
                emit_s(hB, p)
                emit_pv(hA, p)
                if p + 1 < npairs:
                    emit_s(hA, p + 1)
                emit_pv(hB, p)

            # normalize: rows 0..63 unnormalized O^T, row 64 = Z
            # 1/Z on one partition, Pool broadcasts it across the 64 O rows
            rbs = {}
            for h in (hA, hB):
                rb1 = pool.tile([1, 512], F32R, name=f"rb1{h}_{j}", tag="rb1",
                                bufs=2)
                with nc.allow_low_precision(reason="fp32r softmax denom"):
                    nc.vector.reciprocal(rb1[:], ops[h][64:65, :])
                rbs_t = pool.tile([64, 512], F32R, name=f"rbs{h}_{j}",
                                  tag="rbs", bufs=2)
                nc.gpsimd.partition_broadcast(rbs_t[:], rb1[:])
                rbs[h] = rbs_t
            # all 16 O^T tiles stay live until their chunk's projection
            if otc[mt][j] is None:
                otc[mt][j] = pool.tile([128, 512], BF16, name=f"ot{mt}_{j}",
                                       tag="otc", bufs=16)
            nc.vector.tensor_mul(otc[mt][j][0:64, :], ops[hA][0:64, :],
                                 rbs[hA][:])
            st = pool.tile([64, 512], BF16, name=f"st{hB}_{j}", tag="st",
                           bufs=2)
            nc.vector.tensor_mul(st[:], ops[hB][0:64, :], rbs[hB][:])
            # Pool SWDGE queue: keeps the shift off both HWDGE queues and
            # the Act sequencer
            nc.gpsimd.dma_start(otc[mt][j][64:128, :], st[:])
            if filler:
                filler.pop(0)()

    def emit_wp_loads():
        for m in range(4):
            for n in range(2):
                t_ = pool.tile([128, 512], BF16, name=f"wps{m}_{n}",
                               tag="wp2", bufs=8)
                wps[m][n] = t_
                nc.sync.dma_start(
                    t_[:],
                    wp[128 * m:128 * m + 128, 512 * n:512 * n + 512],
                )

    def emit_proj(j, us=(0, 1, 2, 3), tail=False):
        # yo copies alternate DVE / Pool to spread the drain work; the
        # post-attention tail uses the freed "o" PSUM ring for double slots
        # and the now-idle Act engine for copies
        for u in us:
            t = 4 * j + u
            for n in range(2):
                tag = "o" if (tail and (u + n) % 2 == 1) else "qk"
                ps = psp.tile([128, 512], F32, name=f"yps{t}_{n}", tag=tag,
                              bufs=2)
                for m in range(4):
                    nc.tensor.matmul(
                        ps[:], otc[m][j][:, 128 * u:128 * u + 128],
                        wps[m][n][:],
                        start=(m == 0), stop=(m == 3),
                    )
                yo = pool.tile([128, 512], BF16, name=f"yo{t}_{n}", tag="yo",
                               bufs=4)
                eng = nc.vector if (u + n) % 2 == 0 else nc.gpsimd
                eng.tensor_copy(yo[:], ps[:])
                dma_eng = nc.scalar if (tail and (u + n) % 2 == 1) else nc.sync
                dma_eng.dma_start(
                    yout[128 * t:128 * t + 128, 512 * n:512 * n + 512], yo[:]
                )

    # ---- schedule ----
    # qkv(j+1) is woven between attn(j)'s head-pairs as PE filler while the
    # Act engine catches up on the exp backlog; attn(3) gets the projections.
    def rest(jn, *extra):
        def f():
            emit_qkv(jn, parts="q", sel=(2, 3))
            emit_qkv(jn, parts="k", sel=(2, 3))
            emit_qkv(jn, parts="v", sel=(2, 3))
            for e in extra:
                e()
        return f

    emit_qkv(0)
    load_x(1)
    emit_attn(0, filler=[lambda: emit_qkv(1, parts="q", sel=(0, 1)),
                         lambda: emit_qkv(1, parts="k", sel=(0, 1)),
                         lambda: emit_qkv(1, parts="v", sel=(0, 1)),
                         rest(1, lambda: load_x(2), emit_wp_loads)])
    emit_attn(1, filler=[lambda: emit_qkv(2, parts="q", sel=(0, 1)),
                         lambda: emit_qkv(2, parts="k", sel=(0, 1)),
                         lambda: emit_qkv(2, parts="v", sel=(0, 1)),
                         rest(2, lambda: load_x(3))])
    emit_attn(2, filler=[lambda: emit_qkv(3, parts="q", sel=(0, 1)),
                         lambda: emit_qkv(3, parts="k", sel=(0, 1)),
                         lambda: emit_qkv(3, parts="v", sel=(0, 1)),
                         rest(3)])
    emit_attn(3, filler=[lambda: emit_proj(0, us=(0, 1)),
                         lambda: emit_proj(0, us=(2, 3)),
                         lambda: emit_proj(1, us=(0, 1)),
                         lambda: emit_proj(1, us=(2, 3))])
    emit_proj(2, tail=True)
    emit_proj(3, tail=True)

    for m in range(4):
        qtc[m] = [None] * NJ
        otc[m] = [None] * NJ
    pool.release()
    psp.release()


def build(passes=1):
    key = ("nc", passes)
    if key in _CACHE:
        return _CACHE[key]
    nc = bacc.Bacc("TRN2", target_bir_lowering=False, debug=False,
                   num_devices=N_CORES)
    aps = {
        # fp8 DoubleRow operands, host-packed hl-major so chunk/pair slices
        # stay contiguous: contraction row = 256*pair + 128*s + partition,
        # hl = hi/lo residual half
        "xq": nc.dram_tensor("xq", [128, NJ, 2, NP, 2, 512], F8,
                             kind="ExternalInput").ap(),
        "wq": nc.dram_tensor("wq", [128, 2, NP, 2, CH], F8,
                             kind="ExternalInput").ap(),
        "wk": nc.dram_tensor("wk", [128, 2, NP, 2, CH], F8,
                             kind="ExternalInput").ap(),
        "wv": nc.dram_tensor("wv", [128, 2, NP, 2, CH], F8,
                             kind="ExternalInput").ap(),
        "wp": nc.dram_tensor("wp", [CH, C], BF16, kind="ExternalInput").ap(),
        "bq2": nc.dram_tensor("bq2", [128, 4], F32, kind="ExternalInput").ap(),
        "bk2": nc.dram_tensor("bk2", [128, 4], F32, kind="ExternalInput").ap(),
        "mask": nc.dram_tensor("mask", [128, 128], BF16,
                               kind="ExternalInput").ap(),
        "y": nc.dram_tensor("y", [T, C], BF16, kind="ExternalOutput").ap(),
    }
    with tile.TileContext(nc) as tc:
        for _ in range(passes):
            _emit(nc, tc, aps)
    nc.compile()
    _CACHE[key] = nc
    return nc


def _pack_w(a):
    """[1024, CH] f32 -> [128, 2(hl), NP, 2(s), CH] fp8 DoubleRow pack
    with hi/lo residual split."""
    r = a.reshape(NP, 2, 128, CH)              # (pair, s, p, cols)
    hi = r.astype(FP8)
    lo = (r - hi.astype(np.float32)).astype(FP8)
    out = np.stack([hi, lo], axis=2)           # (pair, s, hl, p, cols)
    return np.ascontiguousarray(out.transpose(3, 2, 0, 1, 4))


def _pack_x(a):
    """[1024, T] f32 -> [128, NJ, 2(hl), NP, 2(s), 512] fp8 DoubleRow pack."""
    r = a.reshape(NP, 2, 128, NJ, 512)         # (pair, s, p, j, tl)
    hi = r.astype(FP8)
    lo = (r - hi.astype(np.float32)).astype(FP8)
    out = np.stack([hi, lo], axis=2)           # (pair, s, hl, p, j, tl)
    return np.ascontiguousarray(out.transpose(3, 4, 2, 0, 1, 5))


def make_in_maps(x, Wq, bq, Wk, bk, Wv, bv, Wp, bp):
    # lower-triangle 0/1 mask for the diagonal 128x128 attention blocks
    s_idx = np.arange(128)[:, None]
    t_idx = np.arange(128)[None, :]
    mask = (s_idx <= t_idx).astype(ml_dtypes.bfloat16)
    in_maps = []
    for c in range(N_CORES):
        b, g = c // 2, c % 2
        cols = slice(CH * g, CH * g + CH)
        in_maps.append({
            "xq": _pack_x(np.ascontiguousarray(x[b].T)),
            "wq": _pack_w(np.ascontiguousarray(Wq[:, cols])),
            "wk": _pack_w(np.ascontiguousarray(Wk[:, cols])),
            "wv": _pack_w(np.ascontiguousarray(Wv[:, cols])),
            "wp": np.ascontiguousarray(Wp[cols, :]).astype(ml_dtypes.bfloat16),
            "bq2": np.ascontiguousarray(bq[cols].reshape(4, 128).T),
            "bk2": np.ascontiguousarray(bk[cols].reshape(4, 128).T),
            "mask": np.ascontiguousarray(mask),
        })
    return in_maps


def kernel(x, Wq, bq, Wk, bk, Wv, bv, Wp, bp):
    # host-side prep is pure numpy; convert in case jax arrays are passed
    x, Wq, bq, Wk, bk, Wv, bv, Wp, bp = (
        np.asarray(a, dtype=np.float32)
        for a in (x, Wq, bq, Wk, bk, Wv, bv, Wp, bp)
    )
    nc = build()
    in_maps = make_in_maps(x, Wq, bq, Wk, bk, Wv, bv, Wp, bp)
    # the axon-proxied device occasionally reports a transient unrecoverable
    # exec state that clears on a fresh attempt; retry rather than fail
    last_err = None
    for _attempt in range(3):
        try:
            res = run_bass_kernel_spmd(nc, in_maps, core_ids=list(range(N_CORES)))
            break
        except Exception as e:  # noqa: BLE001
            last_err = e
            import time as _time
            _time.sleep(5)
    else:
        raise last_err
    corr = (bv @ Wp + bp).astype(np.float32)
    out = np.empty((B, T, C), dtype=np.float32)
    for b in range(B):
        out[b] = (res.results[2 * b]["y"].astype(np.float32)
                  + res.results[2 * b + 1]["y"].astype(np.float32) + corr)
    return out


# revision 28
# speedup vs baseline: 1.0652x; 1.0118x over previous
"""Causal self-attention (B=4, T=2048, C=1024, H=16, D=64) on 8 trn2 cores.

Sharding: data-parallel over B (4) x tensor-parallel over head-halves (2).
Core c handles batch c//2 with heads [8*(c%2), 8*(c%2)+8). Each core emits a
partial projection output [2048, 1024]; host sums the two head-half partials
per batch and adds the (bv @ Wp + bp) correction row.

Device layout highlights:
 - QKV projections run as fp8e4m3 DoubleRow matmuls (2 k-tiles per pass, 0.5
   cycles/row) with a 3-term residual split prepared on the host:
   x@w ~= x_hi@w_hi + x_lo@w_hi + x_hi@w_lo, each operand quantized e4m3.
   This is 3/4 the PE cost of fp32r at ~5e-3 max rel err per GEMM.
 - everything downstream of the QKV psums is bf16: Q^T/K^T/V tiles, exp(S)
   tiles, O^T tiles and Wp. bf16 matmuls run at 1.0 cycles/row at ANY width
   (fp32r pays 4x below 256), halve SBUF footprint and DMA bytes, and cost
   ~0.3% relative error against a 2e-2 budget.
 - S^T = K^T.T @ Q^T keeps softmax denominators computable by an in-matmul
   ones-column (V' has a 65th column of ones -> row 64 of O' = Z)
 - softmax skips max-subtraction (logits are ~N(0,1); exp cannot overflow)
 - causal masking via 0/1 mask multiply on the diagonal-block patterns,
   executed on the Pool engine (otherwise idle)
 - softmax denominators: DVE reciprocal of the Z row + Pool-engine
   partition_broadcast (replaces the former ones-column PE matmul)
 - attention processes two heads in lockstep so the PE never waits on the
   Act engine's exp: S_a, S_b, PV_a, S_a', PV_b, ...; QKV/projection work is
   woven between head-pairs as PE filler while Act drains exps
 - projection runs as 8 K=64 groups against per-head-half [64,512] O^T tiles,
   so both heads' normalize muls write partition-0-based tiles and the old
   partition-shifting SBUF->SBUF DMA disappears
"""

import os
import sys

for _p in ("/opt/trn_rl_repo", "/root/.axon_site/_ro/trn_rl_repo"):
    if os.path.isdir(_p) and _p not in sys.path:
        sys.path.insert(0, _p)

import ml_dtypes
import numpy as np
from concourse import bacc, mybir, tile
from concourse.bass_utils import run_bass_kernel_spmd

N_CORES = 8
B, T, C = 4, 2048, 1024
H, D = 16, 64          # full model heads
HG = 8                 # heads per core (head-group)
CH = HG * D            # 512, per-core qkv width
NT = T // 128          # 16 s-tiles
NJ = T // 512          # 4 t-chunks
NP = C // 256          # 4 DoubleRow contraction pairs
F32 = mybir.dt.float32
F32R = mybir.dt.float32r
BF16 = mybir.dt.bfloat16
F8 = mybir.dt.float8e4
DRM = mybir.MatmulPerfMode.DoubleRow
AF = mybir.ActivationFunctionType
FP8 = ml_dtypes.float8_e4m3

# (x_half, w_half) residual terms; term-major so the hi*hi sweep only needs
# the hi tiles that arrive first, then x-lo (scalar queue) before w-lo (sync)
TERMS = ((0, 0), (1, 0), (0, 1))
NTERM = len(TERMS)

_CACHE = {}


def _emit(nc, tc, aps):
    xq, wq, wk, wv, wp, bq2, bk2, mask, yout = (
        aps["xq"], aps["wq"], aps["wk"], aps["wv"], aps["wp"],
        aps["bq2"], aps["bk2"], aps["mask"], aps["y"],
    )

    pool = tc.alloc_tile_pool(name="pool", bufs=1)
    psp = tc.alloc_tile_pool(name="ps", bufs=1, space="PSUM")

    # ---- persistent tensors ----
    kt = [pool.tile([128, T], BF16, name=f"kt{m}", tag="kt", bufs=4)
          for m in range(4)]
    vp = [pool.tile([128, 520], BF16, name=f"vp{i}", tag="vp", bufs=NT)
          for i in range(NT)]
    # single lower-triangle mask (1{s <= t}) for the diagonal 128x128 blocks
    tri = pool.tile([128, 128], BF16, name="tri", tag="tri", bufs=1)
    bqs = pool.tile([128, 4], F32, name="bqs", tag="bias", bufs=2)
    bks = pool.tile([128, 4], F32, name="bks", tag="bias", bufs=2)
    ones_f = pool.tile([128, 64], F32, name="ones_f", tag="ones_f", bufs=1)

    # fp8 DoubleRow weights: [128, pair, s, 512] hi and lo tiles per matrix
    wqkv = {}
    for nm in ("wq", "wk", "wv"):
        wqkv[nm] = [pool.tile([128, NP, 2, CH], F8, name=f"{nm}{hl}",
                              tag="w8", bufs=6) for hl in range(2)]
    # chunk-0 x, split hi/lo for a fast start; later chunks combined
    x0 = [pool.tile([128, NP, 2, 512], F8, name=f"x0_{hl}", tag="x0", bufs=2)
          for hl in range(2)]
    xcomb = [None] * NJ  # chunks 1..3: [128, hl, pair, s, 512] tiles

    # DMA plan: sync carries wq/wk and all later x chunks; scalar carries the
    # chunk-0 x, biases, mask and wv, then stays idle so the Act engine's
    # sequencer is free once the exp stream spins up.
    nc.sync.dma_start(wqkv["wq"][0][:, 0], wq[:, 0, 0])
    nc.scalar.dma_start(x0[0][:, 0], xq[:, 0, 0, 0])
    nc.sync.dma_start(wqkv["wq"][0][:, 1:4], wq[:, 0, 1:4])
    nc.scalar.dma_start(x0[0][:, 1:4], xq[:, 0, 0, 1:4])
    nc.sync.dma_start(wqkv["wq"][1][:], wq[:, 1])
    nc.scalar.dma_start(x0[1][:], xq[:, 0, 1])
    nc.sync.dma_start(wqkv["wk"][0][:], wk[:, 0])
    nc.scalar.dma_start(wqkv["wv"][0][:], wv[:, 0])
    nc.scalar.dma_start(bqs[:], bq2[:])
    nc.scalar.dma_start(bks[:], bk2[:])
    nc.sync.dma_start(wqkv["wk"][1][:], wk[:, 1])
    nc.scalar.dma_start(tri[:], mask[:])
    nc.scalar.dma_start(wqkv["wv"][1][:], wv[:, 1])
    nc.gpsimd.memset(ones_f[:], 1.0)
    for i in range(NT):
        ocol = vp[i][:, 0:520].rearrange("p (h e) -> p h e", e=65)[:, :, 64:65]
        nc.vector.tensor_copy(ocol, ones_f[:, 0:8].unsqueeze(2))

    def load_x(j):
        xt_t = pool.tile([128, 2, NP, 2, 512], F8, name=f"xt{j}", tag="xt",
                         bufs=2)
        nc.sync.dma_start(xt_t[:], xq[:, j])
        xcomb[j] = xt_t

    def x_ap(j, p, xh, c0, cw):
        if j == 0:
            return x0[xh][:, p, :, c0:c0 + cw]
        return xcomb[j][:, xh, p, :, c0:c0 + cw]

    qtc = [[None] * NJ for _ in range(4)]   # per-chunk Q^T tiles
    otc = [[None] * NJ for _ in range(4)]   # per-chunk O^T tiles
    wps = [[None, None] for _ in range(4)]  # wp [128,512] halves, loaded late

    def _qkv_psum(idx, j, nm):
        # chunk 0 runs all four blocks of a part concurrently, term-staged,
        # so the PE can sweep the hi*hi term as soon as the hi tiles land;
        # the sp-tag PSUM slots are idle during chunk 0, borrow two of them
        if j == 0 and idx >= 2:
            return psp.tile([128, 1024], F32, name=nm, tag="sp",
                            bufs=2)[:, 0:512]
        return psp.tile([128, 512], F32, name=nm, tag="qk", bufs=2)[:]

    def emit_qkv(j, parts="qkv", sel=(0, 1, 2, 3)):
        # Q^T and K^T: out [128 qk-dims, 512 t] per m-block
        for part in parts:
            if part in "qk":
                wsrc = wqkv["wq"] if part == "q" else wqkv["wk"]
                bias_t = bqs if part == "q" else bks
                pss = [_qkv_psum(i, j, f"{part}ps{j}_{m}")
                       for i, m in enumerate(sel)]
                for ti, (xh, wh) in enumerate(TERMS):
                    for i, m in enumerate(sel):
                        for u in range(2):
                            for p in range(NP):
                                nc.tensor.matmul(
                                    pss[i][:, 256 * u:256 * u + 256],
                                    wsrc[wh][:, p, :, 128 * m:128 * m + 128],
                                    x_ap(j, p, xh, 256 * u, 256),
                                    start=(ti == 0 and p == 0),
                                    stop=(ti == NTERM - 1 and p == NP - 1),
                                    perf_mode=DRM,
                                )
                for i, m in enumerate(sel):
                    if part == "q":
                        t_ = pool.tile([128, 512], BF16, name=f"qt{m}_{j}",
                                       tag="qtc", bufs=8)
                        qtc[m][j] = t_
                        out_ap = t_[:]
                    else:
                        out_ap = kt[m][:, 512 * j:512 * j + 512]
                    nc.vector.tensor_scalar_add(out_ap, pss[i],
                                                bias_t[:, m:m + 1])
            else:
                # V: out [128 t-slice, 512 v-dims]
                pss = [_qkv_psum(i, j, f"vps{4 * j + u}")
                       for i, u in enumerate(sel)]
                for ti, (xh, wh) in enumerate(TERMS):
                    for i, u in enumerate(sel):
                        for h2 in range(2):
                            for p in range(NP):
                                nc.tensor.matmul(
                                    pss[i][:, 256 * h2:256 * h2 + 256],
                                    x_ap(j, p, xh, 128 * u, 128),
                                    wqkv["wv"][wh][:, p, :,
                                                   256 * h2:256 * h2 + 256],
                                    start=(ti == 0 and p == 0),
                                    stop=(ti == NTERM - 1 and p == NP - 1),
                                    perf_mode=DRM,
                                )
                for i, u in enumerate(sel):
                    dst = vp[4 * j + u][:, 0:520].rearrange(
                        "p (h e) -> p h e", e=65)[:, :, 0:64]
                    src = pss[i].rearrange("p (h e) -> p h e", e=64)
                    nc.vector.tensor_copy(dst, src)

    def tile_layout(p, j):
        # pairs of s-tiles per [128,1024] PSUM slot; diagonal tiles are
        # narrowed to the causally valid t-range [128r, 512).
        # entries: (i, slot_col, valid_t0, width, diag_block_col)
        i0, i1 = 2 * p, 2 * p + 1
        r0_, r1_ = i0 - 4 * j, i1 - 4 * j
        if r1_ < 0:
            return [(i0, 0, 0, 512, None), (i1, 512, 0, 512, None)], 1024
        if r0_ == 0:
            return [(i0, 0, 0, 512, 0), (i1, 512, 128, 384, 512)], 896
        return [(i0, 0, 256, 256, 0), (i1, 256, 384, 128, 256)], 384

    def emit_attn(j, mts=(0, 1, 2, 3), filler=None):
        # process the two heads of each mt pair in lockstep: the PE alternates
        # S and PV between the heads, so each head's exp runs while the other
        # head's matmul occupies the PE. `filler` emits PE work between mts
        # while the Act engine catches up on exps.
        n_i = 4 * j + 4
        npairs = n_i // 2
        for mt in mts:
            hA, hB = 2 * mt, 2 * mt + 1

            ops = {}
            ets = {}
            for h in (hA, hB):
                ops[h] = psp.tile([65, 512], F32, name=f"ops{h}_{j}", tag="o",
                                  bufs=2)

            def emit_s(h, p):
                off = 64 * (h % 2)
                layout, exp_hi = tile_layout(p, j)
                sp = psp.tile([128, 1024], F32, name=f"sp{h}_{j}_{p}", tag="sp",
                              bufs=2)
                for (i, scol, t0, w, _) in layout:
                    nc.tensor.matmul(
                        sp[:, scol:scol + w],
                        kt[mt][off:off + 64, 128 * i:128 * i + 128],
                        qtc[mt][j][off:off + 64, t0:t0 + w],
                        start=True, stop=True,
                    )
                et = pool.tile([128, 1024], BF16, name=f"et{h}_{j}_{p}",
                               tag="et", bufs=4)
                nc.scalar.activation(et[:, 0:exp_hi], sp[:, 0:exp_hi], AF.Exp,
                                     scale=0.125)
                for (i, scol, t0, w, dcol) in layout:
                    if dcol is not None:
                        blk = et[:, dcol:dcol + 128]
                        nc.gpsimd.tensor_mul(blk, blk, tri[:])
                ets[h] = (et, layout)

            def emit_pv(h, p):
                et, layout = ets[h]
                for (i, scol, t0, w, _) in layout:
                    nc.tensor.matmul(
                        ops[h][:, t0:t0 + w], vp[i][:, 65 * h:65 * h + 65],
                        et[:, scol:scol + w],
                        start=(i == 0), stop=(i == n_i - 1),
                    )

            # software pipeline across the two heads
            emit_s(hA, 0)
            for p in range(npairs):
                emit_s(hB, p)
                emit_pv(hA, p)
                if p + 1 < npairs:
                    emit_s(hA, p + 1)
                emit_pv(hB, p)

            # normalize: rows 0..63 unnormalized O^T, row 64 = Z
            # 1/Z on one partition, Pool broadcasts it across the 64 O rows
            rbs = {}
            for h in (hA, hB):
                rb1 = pool.tile([1, 512], F32R, name=f"rb1{h}_{j}", tag="rb1",
                                bufs=2)
                with nc.allow_low_precision(reason="fp32r softmax denom"):
                    nc.vector.reciprocal(rb1[:], ops[h][64:65, :])
                rbs_t = pool.tile([64, 512], F32R, name=f"rbs{h}_{j}",
                                  tag="rbs", bufs=2)
                nc.gpsimd.partition_broadcast(rbs_t[:], rb1[:])
                rbs[h] = rbs_t
            # all 16 O^T tiles stay live until their chunk's projection
            if otc[mt][j] is None:
                otc[mt][j] = pool.tile([128, 512], BF16, name=f"ot{mt}_{j}",
                                       tag="otc", bufs=16)
            nc.vector.tensor_mul(otc[mt][j][0:64, :], ops[hA][0:64, :],
                                 rbs[hA][:])
            st = pool.tile([64, 512], BF16, name=f"st{hB}_{j}", tag="st",
                           bufs=2)
            nc.vector.tensor_mul(st[:], ops[hB][0:64, :], rbs[hB][:])
            nc.sync.dma_start(otc[mt][j][64:128, :], st[:])
            if filler:
                filler.pop(0)()

    def emit_wp_loads():
        for m in range(4):
            for n in range(2):
                t_ = pool.tile([128, 512], BF16, name=f"wps{m}_{n}",
                               tag="wp2", bufs=8)
                wps[m][n] = t_
                nc.sync.dma_start(
                    t_[:],
                    wp[128 * m:128 * m + 128, 512 * n:512 * n + 512],
                )

    def emit_proj(j, us=(0, 1, 2, 3), tail=False):
        # yo copies alternate DVE / Pool to spread the drain work; the
        # post-attention tail uses the freed "o" PSUM ring for double slots
        # and the now-idle Act engine for copies
        for u in us:
            t = 4 * j + u
            for n in range(2):
                tag = "o" if (tail and (u + n) % 2 == 1) else "qk"
                ps = psp.tile([128, 512], F32, name=f"yps{t}_{n}", tag=tag,
                              bufs=2)
                for m in range(4):
                    nc.tensor.matmul(
                        ps[:], otc[m][j][:, 128 * u:128 * u + 128],
                        wps[m][n][:],
                        start=(m == 0), stop=(m == 3),
                    )
                yo = pool.tile([128, 512], BF16, name=f"yo{t}_{n}", tag="yo",
                               bufs=4)
                eng = nc.vector if (u + n) % 2 == 0 else nc.gpsimd
                eng.tensor_copy(yo[:], ps[:])
                dma_eng = nc.scalar if (tail and (u + n) % 2 == 1) else nc.sync
                dma_eng.dma_start(
                    yout[128 * t:128 * t + 128, 512 * n:512 * n + 512], yo[:]
                )

    # ---- schedule ----
    # qkv(j+1) is woven between attn(j)'s head-pairs as PE filler while the
    # Act engine catches up on the exp backlog; attn(3) gets the projections.
    def rest(jn, *extra):
        def f():
            emit_qkv(jn, parts="q", sel=(2, 3))
            emit_qkv(jn, parts="k", sel=(2, 3))
            emit_qkv(jn, parts="v", sel=(2, 3))
            for e in extra:
                e()
        return f

    emit_qkv(0)
    load_x(1)
    emit_attn(0, filler=[lambda: emit_qkv(1, parts="q", sel=(0, 1)),
                         lambda: emit_qkv(1, parts="k", sel=(0, 1)),
                         lambda: emit_qkv(1, parts="v", sel=(0, 1)),
                         rest(1, lambda: load_x(2), emit_wp_loads)])
    emit_attn(1, filler=[lambda: emit_qkv(2, parts="q", sel=(0, 1)),
                         lambda: emit_qkv(2, parts="k", sel=(0, 1)),
                         lambda: emit_qkv(2, parts="v", sel=(0, 1)),
                         rest(2, lambda: load_x(3))])
    emit_attn(2, filler=[lambda: emit_qkv(3, parts="q", sel=(0, 1)),
                         lambda: emit_qkv(3, parts="k", sel=(0, 1)),
                         lambda: emit_qkv(3, parts="v", sel=(0, 1)),
                         rest(3)])
    emit_attn(3, filler=[lambda: emit_proj(0, us=(0, 1)),
                         lambda: emit_proj(0, us=(2, 3)),
                         lambda: emit_proj(1, us=(0, 1)),
                         lambda: emit_proj(1, us=(2, 3))])
    emit_proj(2, tail=True)
    emit_proj(3, tail=True)

    for m in range(4):
        qtc[m] = [None] * NJ
        otc[m] = [None] * NJ
    pool.release()
    psp.release()


def build(passes=1):
    key = ("nc", passes)
    if key in _CACHE:
        return _CACHE[key]
    nc = bacc.Bacc("TRN2", target_bir_lowering=False, debug=False,
                   num_devices=N_CORES)
    aps = {
        # fp8 DoubleRow operands, host-packed hl-major so chunk/pair slices
        # stay contiguous: contraction row = 256*pair + 128*s + partition,
        # hl = hi/lo residual half
        "xq": nc.dram_tensor("xq", [128, NJ, 2, NP, 2, 512], F8,
                             kind="ExternalInput").ap(),
        "wq": nc.dram_tensor("wq", [128, 2, NP, 2, CH], F8,
                             kind="ExternalInput").ap(),
        "wk": nc.dram_tensor("wk", [128, 2, NP, 2, CH], F8,
                             kind="ExternalInput").ap(),
        "wv": nc.dram_tensor("wv", [128, 2, NP, 2, CH], F8,
                             kind="ExternalInput").ap(),
        "wp": nc.dram_tensor("wp", [CH, C], BF16, kind="ExternalInput").ap(),
        "bq2": nc.dram_tensor("bq2", [128, 4], F32, kind="ExternalInput").ap(),
        "bk2": nc.dram_tensor("bk2", [128, 4], F32, kind="ExternalInput").ap(),
        "mask": nc.dram_tensor("mask", [128, 128], BF16,
                               kind="ExternalInput").ap(),
        "y": nc.dram_tensor("y", [T, C], BF16, kind="ExternalOutput").ap(),
    }
    with tile.TileContext(nc) as tc:
        for _ in range(passes):
            _emit(nc, tc, aps)
    nc.compile()
    _CACHE[key] = nc
    return nc


def _pack_w(a):
    """[1024, CH] f32 -> [128, 2(hl), NP, 2(s), CH] fp8 DoubleRow pack
    with hi/lo residual split."""
    r = a.reshape(NP, 2, 128, CH)              # (pair, s, p, cols)
    hi = r.astype(FP8)
    lo = (r - hi.astype(np.float32)).astype(FP8)
    out = np.stack([hi, lo], axis=2)           # (pair, s, hl, p, cols)
    return np.ascontiguousarray(out.transpose(3, 2, 0, 1, 4))


def _pack_x(a):
    """[1024, T] f32 -> [128, NJ, 2(hl), NP, 2(s), 512] fp8 DoubleRow pack."""
    r = a.reshape(NP, 2, 128, NJ, 512)         # (pair, s, p, j, tl)
    hi = r.astype(FP8)
    lo = (r - hi.astype(np.float32)).astype(FP8)
    out = np.stack([hi, lo], axis=2)           # (pair, s, hl, p, j, tl)
    return np.ascontiguousarray(out.transpose(3, 4, 2, 0, 1, 5))


def make_in_maps(x, Wq, bq, Wk, bk, Wv, bv, Wp, bp):
    # lower-triangle 0/1 mask for the diagonal 128x128 attention blocks
    s_idx = np.arange(128)[:, None]
    t_idx = np.arange(128)[None, :]
    mask = (s_idx <= t_idx).astype(ml_dtypes.bfloat16)
    in_maps = []
    for c in range(N_CORES):
        b, g = c // 2, c % 2
        cols = slice(CH * g, CH * g + CH)
        in_maps.append({
            "xq": _pack_x(np.ascontiguousarray(x[b].T)),
            "wq": _pack_w(np.ascontiguousarray(Wq[:, cols])),
            "wk": _pack_w(np.ascontiguousarray(Wk[:, cols])),
            "wv": _pack_w(np.ascontiguousarray(Wv[:, cols])),
            "wp": np.ascontiguousarray(Wp[cols, :]).astype(ml_dtypes.bfloat16),
            "bq2": np.ascontiguousarray(bq[cols].reshape(4, 128).T),
            "bk2": np.ascontiguousarray(bk[cols].reshape(4, 128).T),
            "mask": np.ascontiguousarray(mask),
        })
    return in_maps


def kernel(x, Wq, bq, Wk, bk, Wv, bv, Wp, bp):
    # host-side prep is pure numpy; convert in case jax arrays are passed
    x, Wq, bq, Wk, bk, Wv, bv, Wp, bp = (
        np.asarray(a, dtype=np.float32)
        for a in (x, Wq, bq, Wk, bk, Wv, bv, Wp, bp)
    )
    nc = build()
    in_maps = make_in_maps(x, Wq, bq, Wk, bk, Wv, bv, Wp, bp)
    # the axon-proxied device occasionally reports a transient unrecoverable
    # exec state that clears on a fresh attempt; retry rather than fail
    last_err = None
    for _attempt in range(3):
        try:
            res = run_bass_kernel_spmd(nc, in_maps, core_ids=list(range(N_CORES)))
            break
        except Exception as e:  # noqa: BLE001
            last_err = e
            import time as _time
            _time.sleep(5)
    else:
        raise last_err
    corr = (bv @ Wp + bp).astype(np.float32)
    out = np.empty((B, T, C), dtype=np.float32)
    for b in range(B):
        out[b] = (res.results[2 * b]["y"].astype(np.float32)
                  + res.results[2 * b + 1]["y"].astype(np.float32) + corr)
    return out
